# revision 4
# baseline (speedup 1.0000x reference)
"""Trainium2 Bass kernel for nn_AbstractLiquidRecurrent — v3 (merged epilogue).

Same math as v2 (split3, A folded, y'=(y*R+f)/(R+1/tau+f)) but:
  - G=1: ONE merged [128, 64] epilogue per unfold (one tanh, one d-add,
    one 2-op ~2ULP reciprocal, one mul, one slot1 sub, one u mul, one nm
    add) instead of two staggered 32-wide chains that serialized on DVE
    anyway.  Halves the per-op fixed-cost bill and the DVE serial span.
  - ONE z PSUM bank [128,64] per set; ONE inp preload ACT copy per unfold.
  - PE kept warm (HAM K=8/8) by long junk matmuls into the dead GEMM
    prologue bank during the PE idle window of each unfold.
  - State in one two-slot tile yk [128, 2, 64] f32: slot0 = y (w1 pass and
    fused pass read its hi16 by bitcast), slot1 carries y1 in hi16 halves.
  - MM stream per unfold: 16 w1 MMs (gated only on slot0) then 16 fused
    MMs (gated on slot1 written by one GPS sub).
"""

import time as _time

import numpy as np

import concourse.bass as bass
import concourse.tile as tile
from concourse.tile import add_dep_helper
from concourse import bacc, mybir
from concourse.bass_utils import run_bass_kernel_spmd

N = 512
F = 256
KUNF = 6
B, T = 128, 256
NCORES = 8
BLOC = B // NCORES          # 16 batch rows per core
NCH = N // 128              # 4 n-chunks
FCH = F // 128              # 2 f-chunks

f32 = mybir.dt.float32
bf16 = mybir.dt.bfloat16

VERBOSE = True
RECIP = "approx2"           # "approx2" | "fused2" (custom add+seed op, GPS d) | "accurate"
JUNK_MM = 0                 # PE-warming matmuls per unfold (measured: no help)


def _register_custom_recip_ops():
    """Two self-pinned custom DVE ops for the fused reciprocal path:
      ADD_RECIP_SEED_LIQ: out = seed+NR1 of 1/(in0+in1)   (~0.4% rel)
      RECIP_NR2_LIQ:      out = two Newton steps: in1 refined vs d=in0
    Together with a GPS-computed d they give ~2 ULP in 2 DVE ops with the
    d-add folded into the first op (saves one DVE op on the chain)."""
    import numpy as _np
    import concourse.dve_ops as dve_ops_mod
    from concourse.dve_ops import DveOp
    from concourse.dve_spec import (
        Spec, Src0, Src1, C0, C1, AluOp, Bin, lower,
        _has_src1 as has_src1,
    )
    from concourse.dve_uop import DveOpSpec

    if "ADD_RECIP_SEED_LIQ" in dve_ops_mod._SUB_OPCODE_FOR_NAME:
        return (dve_ops_mod.CUSTOM_DVE_OPS_LIQ)  # type: ignore[attr-defined]

    def _ref_seed(in0, in1, c0, c1, c2):
        d = (in0.astype(_np.float32) + in1.astype(_np.float32)).astype(_np.float32)
        nd = (~d.view(_np.int32)).view(_np.float32)
        y0 = (nd * _np.float32(c0)).astype(_np.float32)
        return (y0 * (_np.float32(c1) - d * y0)).astype(_np.float32)

    def _ref_nr2(in0, in1, c0, c1, c2):
        d = in0.astype(_np.float32)
        y = in1.astype(_np.float32)
        y2 = (y * (_np.float32(c0) - d * y)).astype(_np.float32)
        return (y2 * (_np.float32(c0) - d * y2)).astype(_np.float32)

    _d = Src0 + Src1
    _nd = Bin(AluOp.BITWISE_NOT, _d, _d)
    _s = _nd * C0
    seed_spec = Spec(body=_s * (C1 - _d * _s), reference=_ref_seed)
    _y2 = Src1 * (C0 - Src0 * Src1)
    nr2_spec = Spec(body=_y2 * (C0 - Src0 * _y2), reference=_ref_nr2)

    ops = []
    next_row = max(dve_ops_mod._SUB_OPCODE_FOR_NAME.values()) + 1
    for name, spec in (("ADD_RECIP_SEED_LIQ", seed_spec),
                       ("RECIP_NR2_LIQ", nr2_spec)):
        shas = {}
        for ver in ("v3", "v4"):
            try:
                compiled = DveOpSpec(name=name, opcode=next_row,
                                     uops=lower(spec, ver=ver),
                                     rd1_en=has_src1(spec))
                shas[ver] = compiled.sha(ver)
            except Exception:
                pass
        op = DveOp(name, spec, subdim=False, uops_sha=shas)
        dve_ops_mod.OPS.append(op)
        dve_ops_mod.CUSTOM_DVE_SPECS[name] = spec
        dve_ops_mod._SUB_OPCODE_FOR_NAME[name] = next_row
        next_row += 1
        ops.append(op)
    assert next_row <= 0x20, "custom DVE row overflow"
    dve_ops_mod.CUSTOM_DVE_OPS_LIQ = tuple(ops)  # type: ignore[attr-defined]
    return tuple(ops)


def _bf16_split(arr, terms):
    import ml_dtypes
    out = []
    rem = np.asarray(arr, dtype=np.float32).copy()
    for _ in range(terms):
        h = rem.astype(ml_dtypes.bfloat16)
        out.append(np.ascontiguousarray(h))
        rem = rem - h.astype(np.float32)
    return out


def _hi_view(ap):
    p, n = ap.shape
    return ap.bitcast(bf16).rearrange("p (n two) -> p n two", two=2)[:, :, 1]


def build(t_run=T, mm_mode="split3", recip=None, junk=None):
    assert mm_mode == "split3"
    recip = recip or RECIP
    junk = JUNK_MM if junk is None else junk
    if recip == "fused2":
        seed_op, nr2_op = _register_custom_recip_ops()
    t0 = _time.time()
    nc = bacc.Bacc("TRN2", target_bir_lowering=False, debug=False,
                   disable_frame_to_traceback=True)

    W = NCH * BLOC   # 64: merged state width
    TB = min(32, t_run)
    assert t_run % TB == 0
    NBLK = t_run // TB

    w_d = [nc.dram_tensor(f"wrec{j}", [128, NCH * NCH * 128], bf16,
                          kind="ExternalInput").ap() for j in range(2)]
    win_d = nc.dram_tensor("win", [128, FCH * NCH * 128], f32,
                           kind="ExternalInput").ap()
    it_d = nc.dram_tensor("it", [128, FCH * t_run * BLOC], f32,
                          kind="ExternalInput").ap()
    rtb_d = nc.dram_tensor("rtb", [128, t_run * BLOC], f32,
                           kind="ExternalInput").ap()
    invtau_d = nc.dram_tensor("invtau", [128, NCH], f32,
                              kind="ExternalInput").ap()
    bvec_d = nc.dram_tensor("bvec", [1, NCH * 128], f32,
                            kind="ExternalInput").ap()
    yout_d = nc.dram_tensor("yout", [t_run, 128, NCH * BLOC], f32,
                            kind="ExternalOutput").ap()

    with tile.TileContext(nc) as tc:
        import contextlib
        ctx = contextlib.ExitStack()
        with ctx:
            consts = ctx.enter_context(tc.tile_pool(name="consts", bufs=1))
            state = ctx.enter_context(tc.tile_pool(name="state", bufs=5))
            work = ctx.enter_context(tc.tile_pool(name="work", bufs=6))
            prep = ctx.enter_context(tc.tile_pool(name="prep", bufs=4))
            psum = ctx.enter_context(tc.tile_pool(name="psum", bufs=1, space="PSUM"))

            w_sb = []
            for j in range(2):
                wj = consts.tile([128, NCH * NCH * 128], bf16, name=f"w_sb{j}")
                nc.sync.dma_start(wj[:], w_d[j][:])
                w_sb.append(wj)
            win_sb = consts.tile([128, FCH * NCH * 128], f32, name="win_sb")
            nc.sync.dma_start(win_sb[:], win_d[:])
            it_sb = consts.tile([128, FCH * t_run * BLOC], f32, name="it_sb")
            nc.sync.dma_start(it_sb[:], it_d[:])
            rtb_sb = consts.tile([128, t_run * BLOC], f32, name="rtb_sb")
            nc.sync.dma_start(rtb_sb[:], rtb_d[:])
            invtau_sb = consts.tile([128, NCH], f32)
            nc.sync.dma_start(invtau_sb[:], invtau_d[:])
            bvec_sb = consts.tile([1, NCH * 128], f32)
            nc.sync.dma_start(bvec_sb[:], bvec_d[:])
            ones_sb = consts.tile([1, TB * BLOC], f32)
            nc.vector.memset(ones_sb[:], 1.0)
            junk1 = consts.tile([1, W], bf16)
            nc.vector.memset(junk1[:], 0.0)
            junk2 = consts.tile([1, 128], bf16)
            nc.vector.memset(junk2[:], 0.0)
            inp_sb = consts.tile([128, t_run * NCH * BLOC], f32, name="inp_sb")

            # PSUM: z banks (2 sets, merged [128, W]) + GEMM/junk banks
            zb = [psum.tile([128, W], f32, name=f"z{s}", tag=f"z{s}")
                  for s in range(2)]
            pg = [psum.tile([128, TB * BLOC], f32, name=f"pg{q}", tag=f"pg{q}")
                  for q in range(2)]

            _dve_chain = [None]
            _pe_chain = [None]
            _act_chain = [None]
            _gps_chain = [None]

            def _chain(slot, op, why):
                if slot[0] is not None:
                    add_dep_helper(op.ins, slot[0].ins, sync=False, reason=why)
                slot[0] = op
                return op

            def chain_dve(op):
                return _chain(_dve_chain, op, "DVE order")

            def chain_mm(op):
                return _chain(_pe_chain, op, "PE order")

            def chain_act(op):
                return _chain(_act_chain, op, "ACT order")

            def chain_gps(op):
                return _chain(_gps_chain, op, "GPS order")

            # ---- input projection GEMM: inp = i @ Win.T + b ----
            def win_tile(fc, mc):
                off = (fc * NCH + mc) * 128
                return win_sb[:, off:off + 128]

            for mc in range(NCH):
                for tb in range(NBLK):
                    bank = pg[(mc * NBLK + tb) % 2]
                    for fc in range(FCH):
                        base = fc * t_run * BLOC + tb * TB * BLOC
                        chain_mm(nc.tensor.matmul(
                            bank[:],
                            lhsT=win_tile(fc, mc),
                            rhs=it_sb[:, base:base + TB * BLOC],
                            start=(fc == 0), stop=False,
                            skip_group_check=True))
                    chain_mm(nc.tensor.matmul(
                        bank[:],
                        lhsT=bvec_sb[:, mc * 128:(mc + 1) * 128],
                        rhs=ones_sb[:],
                        start=False, stop=True,
                        skip_group_check=True))
                    dst = inp_sb[:].rearrange(
                        "p (t m b) -> p t m b", t=t_run, m=NCH)[
                        :, tb * TB:(tb + 1) * TB, mc, :]
                    chain_act(nc.scalar.activation(
                        dst, bank[:].rearrange("p (t b) -> p t b", t=TB),
                        mybir.ActivationFunctionType.Copy))

            # arm has_written bits of the z banks once
            for s in range(2):
                chain_mm(nc.tensor.matmul(
                    zb[s][:], lhsT=junk2[:], rhs=junk1[:],
                    start=True, stop=True))

            # ---- state: one two-slot tile ----
            def slot0(yk):
                return yk[:].rearrange("p (u b) -> p u b", u=2)[:, 0, :]

            def slot1_hi(yk):
                v = yk[:].bitcast(bf16).rearrange(
                    "p (u b two) -> p u b two", u=2, two=2)
                return v[:, 1, :, 1]

            def fused_rhs(yk, kc):
                v = yk[:].bitcast(bf16).rearrange(
                    "p (u b two) -> p u b two", u=2, two=2)[:, :, :, 1]
                return v[:, :, kc * BLOC:(kc + 1) * BLOC]

            yk_cur = state.tile([128, 2 * W], f32, name="y_init", tag="y")
            nc.vector.memset(yk_cur[:], 0.0)
            u_cur = work.tile([128, W], f32, name="u_init", tag="u")
            nc.vector.memset(u_cur[:], 0.0)

            def rt_slice(t):
                return rtb_sb[:, t * BLOC:(t + 1) * BLOC]

            def mk_rt_exp(t):
                rte = prep.tile([128, W], f32, tag="rte", name=f"rte{t}")
                chain_dve(nc.vector.tensor_copy(
                    rte[:].rearrange("p (m b) -> p m b", m=NCH),
                    rt_slice(t).unsqueeze(1).broadcast_to([128, NCH, BLOC])))
                return rte

            def mk_p2(t, rte):
                p2 = prep.tile([128, W], f32, tag="p2", name=f"p2_{t}")
                chain_dve(nc.vector.tensor_add(
                    p2[:].rearrange("p (m b) -> p m b", m=NCH),
                    rte[:].rearrange("p (m b) -> p m b", m=NCH),
                    invtau_sb[:, :].unsqueeze(2).broadcast_to([128, NCH, BLOC]),
                ))
                return p2

            def preload(bank, t2):
                src = inp_sb[:, t2 * W:(t2 + 1) * W]
                chain_act(nc.scalar.activation(
                    bank[:], src,
                    mybir.ActivationFunctionType.Copy))

            rte_cur = mk_rt_exp(0)
            p2_cur = mk_p2(0, rte_cur)
            preload(zb[0], 0)
            # unfold precision schedule: k0-k2 recompute z = W@y + inp in
            # full split3 (32 tiles); k3-k5 accumulate z += w0 @ hi16(dy)
            # (16 tiles) onto the PSUM-persistent z -- dy = y_k - y_{k-1} is
            # ~4% of |y| so a single bf16 pass holds the error budget.
            MODE = ("full", "full", "full", "single", "single", "single")

            def w_tile(j, kc, mc):
                off = (kc * NCH + mc) * 128
                return w_sb[j][:, off:off + 128]

            junk_rhs = w_sb[0][:, 0:512]

            def emit_junk():
                for _ in range(junk):
                    chain_mm(nc.tensor.matmul(
                        pg[0][:], lhsT=w_tile(0, 0, 0), rhs=junk_rhs,
                        start=True, stop=True, skip_group_check=True))

            rte_nxt = None
            p2_nxt = None
            dlt_cur = None
            total_gk = t_run * KUNF
            for t in range(t_run):
                for k in range(KUNF):
                    gk = t * KUNF + k
                    last_unfold = (k == KUNF - 1)
                    bank = zb[t % 2]
                    mode = MODE[k]
                    next_mode = MODE[(k + 1) % KUNF]

                    # re-preload inp for the k1/k2 full recomputes (ACT chain
                    # puts the copy after tanh(k-1)); t+1's bank at k==4
                    if k in (1, 2):
                        preload(bank, t)
                    if k == 4 and t + 1 < t_run:
                        preload(zb[(t + 1) % 2], t + 1)

                    # allocate all epilogue tiles up-front so their pool
                    # WAR-guard sems dispatch early and pre-satisfy
                    f_t = work.tile([128, W], f32, name="f", tag="f")
                    d_t = work.tile([128, W], f32, name="d", tag="d")
                    r_t = work.tile([128, W], f32, name="r", tag="r")
                    scr = work.tile([128, W], f32, name="rs", tag="rs")
                    nm_t = work.tile([128, W], f32, name="nm", tag="nm")
                    yk_new = state.tile([128, 2 * W], f32, name="y_n", tag="y")
                    u_new = work.tile([128, W], f32, name="u_n", tag="u")

                    # ---- MM stream ----
                    cnt = 0
                    total = 2 * NCH * NCH if mode == "full" else NCH * NCH
                    first_mm = None
                    last_mm = None

                    def emit(mm):
                        nonlocal first_mm, last_mm
                        if first_mm is None:
                            first_mm = mm
                        last_mm = mm

                    if mode == "full":
                        hv0 = _hi_view(slot0(yk_cur))
                        for kc in range(NCH):
                            for mc in range(NCH):
                                cnt += 1
                                emit(nc.tensor.matmul(
                                    bank[:, mc * BLOC:(mc + 1) * BLOC],
                                    lhsT=w_tile(1, kc, mc),
                                    rhs=hv0[:, kc * BLOC:(kc + 1) * BLOC],
                                    start=False, stop=(cnt == total),
                                    skip_group_check=True))
                        for kc in range(NCH):
                            frhs = fused_rhs(yk_cur, kc)
                            for mc in range(NCH):
                                cnt += 1
                                emit(nc.tensor.matmul(
                                    bank[:, mc * BLOC:(mc + 1) * BLOC]
                                    .unsqueeze(1).broadcast_to([128, 2, BLOC]),
                                    lhsT=w_tile(0, kc, mc),
                                    rhs=frhs,
                                    start=False, stop=(cnt == total),
                                    skip_group_check=True))
                    else:
                        hvD = _hi_view(dlt_cur[:])
                        for kc in range(NCH):
                            for mc in range(NCH):
                                cnt += 1
                                emit(nc.tensor.matmul(
                                    bank[:, mc * BLOC:(mc + 1) * BLOC],
                                    lhsT=w_tile(0, kc, mc),
                                    rhs=hvD[:, kc * BLOC:(kc + 1) * BLOC],
                                    start=False, stop=(cnt == total),
                                    skip_group_check=True))
                    if _pe_chain[0] is not None:
                        add_dep_helper(first_mm.ins, _pe_chain[0].ins,
                                       sync=False, reason="PE ordering")
                    _pe_chain[0] = last_mm

                    # ---- merged epilogue ----
                    chain_act(nc.scalar.activation(
                        f_t[:], bank[:],
                        mybir.ActivationFunctionType.Tanh))

                    if recip == "fused2":
                        # pool WAR-guard lands on this early touch (runs in
                        # the tanh window) instead of stalling the mul
                        chain_dve(nc.vector.memset(yk_new[:, W:W + 1], 0.0))
                        # DVE: seed(f+p2) -> [nm fills d-wait] -> NR2 -> mul
                        # GPS computes d = f + p2 in parallel for the NR.
                        chain_gps(nc.gpsimd.tensor_add(
                            d_t[:], f_t[:], p2_cur[:]))
                        chain_dve(nc.vector._custom_dve(
                            seed_op, out=scr[:], in0=f_t[:], in1=p2_cur[:],
                            s0=-0.23549792, s1=2.0017324))
                        chain_dve(nc.vector.tensor_add(
                            nm_t[:], f_t[:], u_cur[:]))
                        chain_dve(nc.vector._custom_dve(
                            nr2_op, out=r_t[:], in0=d_t[:], in1=scr[:],
                            s0=2.0))
                    else:
                        chain_dve(nc.vector.tensor_add(
                            d_t[:], f_t[:], p2_cur[:]))
                        if recip == "accurate":
                            chain_dve(nc.vector.reciprocal(
                                out=r_t[:], in_=d_t[:]))
                        else:
                            chain_dve(nc.vector.reciprocal_approx_fast(
                                out=scr[:], in_=d_t[:]))
                            from concourse.dve_ops import RECIPROCAL_APPROX_NR
                            chain_dve(nc.vector._custom_dve(
                                RECIPROCAL_APPROX_NR, out=r_t[:], in0=d_t[:],
                                in1=scr[:], s0=2.0))
                        chain_gps(nc.gpsimd.tensor_add(
                            nm_t[:], f_t[:], u_cur[:]))
                    # split mul (and the follow-up sub) into kc-halves: the
                    # next stream's kc0-1 MMs gate on the _a half only
                    H = W // 2
                    s0n = slot0(yk_new)
                    s1h = slot1_hi(yk_new)
                    s0o = slot0(yk_cur)
                    if next_mode == "single":
                        dlt_new = work.tile([128, W], f32, name="dd", tag="dd")
                        chain_dve(nc.vector.tensor_mul(
                            s0n[:, 0:H], nm_t[:, 0:H], r_t[:, 0:H]))
                        chain_dve(nc.vector.tensor_sub(
                            dlt_new[:, 0:H], s0n[:, 0:H], s0o[:, 0:H]))
                        chain_dve(nc.vector.tensor_mul(
                            s0n[:, H:], nm_t[:, H:], r_t[:, H:]))
                        chain_dve(nc.vector.tensor_sub(
                            dlt_new[:, H:], s0n[:, H:], s0o[:, H:]))
                        dlt_cur = dlt_new
                    else:
                        chain_dve(nc.vector.tensor_mul(
                            s0n[:, 0:H], nm_t[:, 0:H], r_t[:, 0:H]))
                        chain_dve(nc.vector.tensor_mul(
                            s0n[:, H:], nm_t[:, H:], r_t[:, H:]))
                        chain_gps(nc.gpsimd.tensor_sub(
                            s1h[:, 0:H], s0n[:, 0:H], _hi_view(s0n)[:, 0:H]))
                        chain_gps(nc.gpsimd.tensor_sub(
                            s1h[:, H:], s0n[:, H:], _hi_view(s0n)[:, H:]))
                    if not last_unfold:
                        chain_gps(nc.gpsimd.tensor_mul(
                            u_new[:], s0n, rte_cur[:]))
                    elif t + 1 < t_run:
                        chain_gps(nc.gpsimd.tensor_mul(
                            u_new[:], s0n, rte_nxt[:]))

                    if gk + 1 < total_gk:
                        emit_junk()

                    if k == 2 and t + 1 < t_run:
                        rte_nxt = mk_rt_exp(t + 1)
                        p2_nxt = mk_p2(t + 1, rte_nxt)
                    yk_cur = yk_new
                    u_cur = u_new

                nc.sync.dma_start(yout_d[t][:, :], slot0(yk_cur))
                if t + 1 < t_run:
                    rte_cur, p2_cur = rte_nxt, p2_nxt

    t1 = _time.time()
    nc.compile()
    t2 = _time.time()
    if VERBOSE:
        print(f"[build] trace+schedule {t1-t0:.1f}s, bacc compile {t2-t1:.1f}s",
              flush=True)
    return nc


def _host_prep(i, delta_t, W_rec, W_in, b, A, tau, t_run):
    i = np.asarray(i, dtype=np.float32)
    delta_t = np.asarray(delta_t, dtype=np.float32)
    W_rec = np.asarray(W_rec, dtype=np.float32)
    W_in = np.asarray(W_in, dtype=np.float32)
    b = np.asarray(b, dtype=np.float32)
    A = np.asarray(A, dtype=np.float32)
    tau = np.asarray(tau, dtype=np.float32)

    def tiles_rec(m):
        return m.reshape(NCH, 128, NCH, 128).transpose(1, 0, 2, 3).reshape(128, -1)

    def tiles_in(m):
        return m.reshape(FCH, 128, NCH, 128).transpose(1, 0, 2, 3).reshape(128, -1)

    Wt = (W_rec * A[None, :]).T
    w_arrs = [np.ascontiguousarray(tiles_rec(x.astype(np.float32)).astype(x.dtype))
              for x in _bf16_split(Wt, 2)]
    win_arr = np.ascontiguousarray(tiles_in(W_in.T), dtype=np.float32)

    invtau = np.ascontiguousarray((1.0 / tau).reshape(NCH, 128).T, dtype=np.float32)
    bvec = np.ascontiguousarray(b.reshape(1, -1), dtype=np.float32)

    in_maps = []
    for c in range(NCORES):
        bsl = slice(c * BLOC, (c + 1) * BLOC)
        ii = i[bsl, :t_run]
        it = np.ascontiguousarray(
            ii.reshape(BLOC, t_run, FCH, 128).transpose(3, 2, 1, 0)
            .reshape(128, -1), dtype=np.float32)
        r = (KUNF / np.maximum(delta_t[bsl, :t_run], 1e-30)).T.reshape(1, -1)
        rtb = np.ascontiguousarray(
            np.broadcast_to(r, (128, r.shape[1])), dtype=np.float32)
        m = {"it": it, "rtb": rtb, "invtau": invtau, "bvec": bvec,
             "win": win_arr, "wrec0": w_arrs[0], "wrec1": w_arrs[1]}
        in_maps.append(m)
    return in_maps


def _host_unshard(results, A, t_run):
    A = np.asarray(A, dtype=np.float32)
    out = np.empty((B, t_run, N), dtype=np.float32)
    for c in range(NCORES):
        y = results[c]["yout"].reshape(t_run, 128, NCH, BLOC)
        xc = y.transpose(3, 0, 2, 1).reshape(BLOC, t_run, N)
        out[c * BLOC:(c + 1) * BLOC] = xc * A[None, None, :]
    return out


_BUILD_CACHE = {}


def _get_built(t_run, mm_mode):
    key = (t_run, mm_mode)
    if key not in _BUILD_CACHE:
        _BUILD_CACHE[key] = build(t_run, mm_mode)
    return _BUILD_CACHE[key]


def run(i, delta_t, W_rec, W_in, b, A, tau, t_run=T, mm_mode="split3",
        **rb_kwargs):
    nc = _get_built(t_run, mm_mode)
    in_maps = _host_prep(i, delta_t, W_rec, W_in, b, A, tau, t_run)
    res = run_bass_kernel_spmd(nc, in_maps, list(range(NCORES)), **rb_kwargs)
    out = _host_unshard(res.results, A, t_run)
    return out, res


MM_DTYPE = "split3"


def kernel(i, delta_t, W_rec, W_in, b, A, tau):
    out, _ = run(i, delta_t, W_rec, W_in, b, A, tau)
    return out


# revision 5
# speedup vs baseline: 1.1732x; 1.1732x over previous
"""Trainium2 Bass kernel for nn_AbstractLiquidRecurrent — v3 (merged epilogue).

Same math as v2 (split3, A folded, y'=(y*R+f)/(R+1/tau+f)) but:
  - G=1: ONE merged [128, 64] epilogue per unfold (one tanh, one d-add,
    one 2-op ~2ULP reciprocal, one mul, one slot1 sub, one u mul, one nm
    add) instead of two staggered 32-wide chains that serialized on DVE
    anyway.  Halves the per-op fixed-cost bill and the DVE serial span.
  - ONE z PSUM bank [128,64] per set; ONE inp preload ACT copy per unfold.
  - PE kept warm (HAM K=8/8) by long junk matmuls into the dead GEMM
    prologue bank during the PE idle window of each unfold.
  - State in one two-slot tile yk [128, 2, 64] f32: slot0 = y (w1 pass and
    fused pass read its hi16 by bitcast), slot1 carries y1 in hi16 halves.
  - MM stream per unfold: 16 w1 MMs (gated only on slot0) then 16 fused
    MMs (gated on slot1 written by one GPS sub).
"""

import time as _time

import numpy as np

import concourse.bass as bass
import concourse.tile as tile
from concourse.tile import add_dep_helper
from concourse import bacc, mybir
from concourse.bass_utils import run_bass_kernel_spmd

N = 512
F = 256
KUNF = 6
B, T = 128, 256
NCORES = 8
BLOC = B // NCORES          # 16 batch rows per core
NCH = N // 128              # 4 n-chunks
FCH = F // 128              # 2 f-chunks

f32 = mybir.dt.float32
bf16 = mybir.dt.bfloat16

VERBOSE = True
RECIP = "approx2"           # "approx2" | "fused2" (custom add+seed op, GPS d) | "accurate"
JUNK_MM = 0                 # PE-warming matmuls per unfold (measured: no help)


def _register_custom_recip_ops():
    """Two self-pinned custom DVE ops for the fused reciprocal path:
      ADD_RECIP_SEED_LIQ: out = seed+NR1 of 1/(in0+in1)   (~0.4% rel)
      RECIP_NR2_LIQ:      out = two Newton steps: in1 refined vs d=in0
    Together with a GPS-computed d they give ~2 ULP in 2 DVE ops with the
    d-add folded into the first op (saves one DVE op on the chain)."""
    import numpy as _np
    import concourse.dve_ops as dve_ops_mod
    from concourse.dve_ops import DveOp
    from concourse.dve_spec import (
        Spec, Src0, Src1, C0, C1, AluOp, Bin, lower,
        _has_src1 as has_src1,
    )
    from concourse.dve_uop import DveOpSpec

    if "ADD_RECIP_SEED_LIQ" in dve_ops_mod._SUB_OPCODE_FOR_NAME:
        return (dve_ops_mod.CUSTOM_DVE_OPS_LIQ)  # type: ignore[attr-defined]

    def _ref_seed(in0, in1, c0, c1, c2):
        d = (in0.astype(_np.float32) + in1.astype(_np.float32)).astype(_np.float32)
        nd = (~d.view(_np.int32)).view(_np.float32)
        y0 = (nd * _np.float32(c0)).astype(_np.float32)
        return (y0 * (_np.float32(c1) - d * y0)).astype(_np.float32)

    def _ref_nr2(in0, in1, c0, c1, c2):
        d = in0.astype(_np.float32)
        y = in1.astype(_np.float32)
        y2 = (y * (_np.float32(c0) - d * y)).astype(_np.float32)
        return (y2 * (_np.float32(c0) - d * y2)).astype(_np.float32)

    _d = Src0 + Src1
    _nd = Bin(AluOp.BITWISE_NOT, _d, _d)
    _s = _nd * C0
    seed_spec = Spec(body=_s * (C1 - _d * _s), reference=_ref_seed)
    _y2 = Src1 * (C0 - Src0 * Src1)
    nr2_spec = Spec(body=_y2 * (C0 - Src0 * _y2), reference=_ref_nr2)

    ops = []
    next_row = max(dve_ops_mod._SUB_OPCODE_FOR_NAME.values()) + 1
    for name, spec in (("ADD_RECIP_SEED_LIQ", seed_spec),
                       ("RECIP_NR2_LIQ", nr2_spec)):
        shas = {}
        for ver in ("v3", "v4"):
            try:
                compiled = DveOpSpec(name=name, opcode=next_row,
                                     uops=lower(spec, ver=ver),
                                     rd1_en=has_src1(spec))
                shas[ver] = compiled.sha(ver)
            except Exception:
                pass
        op = DveOp(name, spec, subdim=False, uops_sha=shas)
        dve_ops_mod.OPS.append(op)
        dve_ops_mod.CUSTOM_DVE_SPECS[name] = spec
        dve_ops_mod._SUB_OPCODE_FOR_NAME[name] = next_row
        next_row += 1
        ops.append(op)
    assert next_row <= 0x20, "custom DVE row overflow"
    dve_ops_mod.CUSTOM_DVE_OPS_LIQ = tuple(ops)  # type: ignore[attr-defined]
    return tuple(ops)


def _bf16_split(arr, terms):
    import ml_dtypes
    out = []
    rem = np.asarray(arr, dtype=np.float32).copy()
    for _ in range(terms):
        h = rem.astype(ml_dtypes.bfloat16)
        out.append(np.ascontiguousarray(h))
        rem = rem - h.astype(np.float32)
    return out


def _hi_view(ap):
    p, n = ap.shape
    return ap.bitcast(bf16).rearrange("p (n two) -> p n two", two=2)[:, :, 1]


def build(t_run=T, mm_mode="split3", recip=None, junk=None):
    assert mm_mode == "split3"
    recip = recip or RECIP
    junk = JUNK_MM if junk is None else junk
    if recip == "fused2":
        seed_op, nr2_op = _register_custom_recip_ops()
    t0 = _time.time()
    nc = bacc.Bacc("TRN2", target_bir_lowering=False, debug=False,
                   disable_frame_to_traceback=True)

    W = NCH * BLOC   # 64: merged state width
    TB = min(32, t_run)
    assert t_run % TB == 0
    NBLK = t_run // TB

    w_d = [nc.dram_tensor(f"wrec{j}", [128, NCH * NCH * 128], bf16,
                          kind="ExternalInput").ap() for j in range(2)]
    win_d = nc.dram_tensor("win", [128, FCH * NCH * 128], f32,
                           kind="ExternalInput").ap()
    it_d = nc.dram_tensor("it", [128, FCH * t_run * BLOC], f32,
                          kind="ExternalInput").ap()
    rtb_d = nc.dram_tensor("rtb", [128, t_run * BLOC], f32,
                           kind="ExternalInput").ap()
    invtau_d = nc.dram_tensor("invtau", [128, NCH], f32,
                              kind="ExternalInput").ap()
    bvec_d = nc.dram_tensor("bvec", [1, NCH * 128], f32,
                            kind="ExternalInput").ap()
    yout_d = nc.dram_tensor("yout", [t_run, 128, NCH * BLOC], f32,
                            kind="ExternalOutput").ap()

    with tile.TileContext(nc) as tc:
        import contextlib
        ctx = contextlib.ExitStack()
        with ctx:
            consts = ctx.enter_context(tc.tile_pool(name="consts", bufs=1))
            state = ctx.enter_context(tc.tile_pool(name="state", bufs=5))
            work = ctx.enter_context(tc.tile_pool(name="work", bufs=6))
            prep = ctx.enter_context(tc.tile_pool(name="prep", bufs=4))
            psum = ctx.enter_context(tc.tile_pool(name="psum", bufs=1, space="PSUM"))

            w_sb = []
            for j in range(2):
                wj = consts.tile([128, NCH * NCH * 128], bf16, name=f"w_sb{j}")
                nc.sync.dma_start(wj[:], w_d[j][:])
                w_sb.append(wj)
            win_sb = consts.tile([128, FCH * NCH * 128], f32, name="win_sb")
            nc.sync.dma_start(win_sb[:], win_d[:])
            it_sb = consts.tile([128, FCH * t_run * BLOC], f32, name="it_sb")
            nc.sync.dma_start(it_sb[:], it_d[:])
            rtb_sb = consts.tile([128, t_run * BLOC], f32, name="rtb_sb")
            nc.sync.dma_start(rtb_sb[:], rtb_d[:])
            invtau_sb = consts.tile([128, NCH], f32)
            nc.sync.dma_start(invtau_sb[:], invtau_d[:])
            bvec_sb = consts.tile([1, NCH * 128], f32)
            nc.sync.dma_start(bvec_sb[:], bvec_d[:])
            ones_sb = consts.tile([1, TB * BLOC], f32)
            nc.vector.memset(ones_sb[:], 1.0)
            junk1 = consts.tile([1, W], bf16)
            nc.vector.memset(junk1[:], 0.0)
            junk2 = consts.tile([1, 128], bf16)
            nc.vector.memset(junk2[:], 0.0)
            inp_sb = consts.tile([128, t_run * NCH * BLOC], f32, name="inp_sb")

            # PSUM: z banks (2 sets, merged [128, W]) + GEMM/junk banks
            zb = [psum.tile([128, W], f32, name=f"z{s}", tag=f"z{s}")
                  for s in range(2)]
            pg = [psum.tile([128, TB * BLOC], f32, name=f"pg{q}", tag=f"pg{q}")
                  for q in range(2)]

            _dve_chain = [None]
            _pe_chain = [None]
            _act_chain = [None]
            _gps_chain = [None]

            def _chain(slot, op, why):
                if slot[0] is not None:
                    add_dep_helper(op.ins, slot[0].ins, sync=False, reason=why)
                slot[0] = op
                return op

            def chain_dve(op):
                return _chain(_dve_chain, op, "DVE order")

            def chain_mm(op):
                return _chain(_pe_chain, op, "PE order")

            def chain_act(op):
                return _chain(_act_chain, op, "ACT order")

            def chain_gps(op):
                return _chain(_gps_chain, op, "GPS order")

            # ---- input projection GEMM: inp = i @ Win.T + b ----
            def win_tile(fc, mc):
                off = (fc * NCH + mc) * 128
                return win_sb[:, off:off + 128]

            for mc in range(NCH):
                for tb in range(NBLK):
                    bank = pg[(mc * NBLK + tb) % 2]
                    for fc in range(FCH):
                        base = fc * t_run * BLOC + tb * TB * BLOC
                        chain_mm(nc.tensor.matmul(
                            bank[:],
                            lhsT=win_tile(fc, mc),
                            rhs=it_sb[:, base:base + TB * BLOC],
                            start=(fc == 0), stop=False,
                            skip_group_check=True))
                    chain_mm(nc.tensor.matmul(
                        bank[:],
                        lhsT=bvec_sb[:, mc * 128:(mc + 1) * 128],
                        rhs=ones_sb[:],
                        start=False, stop=True,
                        skip_group_check=True))
                    dst = inp_sb[:].rearrange(
                        "p (t m b) -> p t m b", t=t_run, m=NCH)[
                        :, tb * TB:(tb + 1) * TB, mc, :]
                    chain_act(nc.scalar.activation(
                        dst, bank[:].rearrange("p (t b) -> p t b", t=TB),
                        mybir.ActivationFunctionType.Copy))

            # arm has_written bits of the z banks once
            for s in range(2):
                chain_mm(nc.tensor.matmul(
                    zb[s][:], lhsT=junk2[:], rhs=junk1[:],
                    start=True, stop=True))

            # ---- state: one two-slot tile ----
            def slot0(yk):
                return yk[:].rearrange("p (u b) -> p u b", u=2)[:, 0, :]

            def slot1_hi(yk):
                v = yk[:].bitcast(bf16).rearrange(
                    "p (u b two) -> p u b two", u=2, two=2)
                return v[:, 1, :, 1]

            def fused_rhs(yk, kc):
                v = yk[:].bitcast(bf16).rearrange(
                    "p (u b two) -> p u b two", u=2, two=2)[:, :, :, 1]
                return v[:, :, kc * BLOC:(kc + 1) * BLOC]

            yk_cur = state.tile([128, 2 * W], f32, name="y_init", tag="y")
            nc.vector.memset(yk_cur[:], 0.0)
            u_cur = work.tile([128, W], f32, name="u_init", tag="u")
            nc.vector.memset(u_cur[:], 0.0)

            def rt_slice(t):
                return rtb_sb[:, t * BLOC:(t + 1) * BLOC]

            def mk_rt_exp(t):
                rte = prep.tile([128, W], f32, tag="rte", name=f"rte{t}")
                chain_dve(nc.vector.tensor_copy(
                    rte[:].rearrange("p (m b) -> p m b", m=NCH),
                    rt_slice(t).unsqueeze(1).broadcast_to([128, NCH, BLOC])))
                return rte

            def mk_p2(t, rte):
                p2 = prep.tile([128, W], f32, tag="p2", name=f"p2_{t}")
                chain_dve(nc.vector.tensor_add(
                    p2[:].rearrange("p (m b) -> p m b", m=NCH),
                    rte[:].rearrange("p (m b) -> p m b", m=NCH),
                    invtau_sb[:, :].unsqueeze(2).broadcast_to([128, NCH, BLOC]),
                ))
                return p2

            def preload(gk):
                t2 = gk // KUNF
                src = inp_sb[:, t2 * W:(t2 + 1) * W]
                chain_act(nc.scalar.activation(
                    zb[gk % 2][:], src,
                    mybir.ActivationFunctionType.Copy))

            rte_cur = mk_rt_exp(0)
            p2_cur = mk_p2(0, rte_cur)
            preload(0)

            def w_tile(j, kc, mc):
                off = (kc * NCH + mc) * 128
                return w_sb[j][:, off:off + 128]

            junk_rhs = w_sb[0][:, 0:512]

            def emit_junk():
                for _ in range(junk):
                    chain_mm(nc.tensor.matmul(
                        pg[0][:], lhsT=w_tile(0, 0, 0), rhs=junk_rhs,
                        start=True, stop=True, skip_group_check=True))

            rte_nxt = None
            p2_nxt = None
            total_gk = t_run * KUNF
            for t in range(t_run):
                for k in range(KUNF):
                    gk = t * KUNF + k
                    s = gk % 2
                    last_unfold = (k == KUNF - 1)

                    if gk + 1 < total_gk:
                        preload(gk + 1)

                    # allocate all epilogue tiles up-front so their pool
                    # WAR-guard sems dispatch early and pre-satisfy
                    f_t = work.tile([128, W], f32, name="f", tag="f")
                    d_t = work.tile([128, W], f32, name="d", tag="d")
                    r_t = work.tile([128, W], f32, name="r", tag="r")
                    scr = work.tile([128, W], f32, name="rs", tag="rs")
                    nm_t = work.tile([128, W], f32, name="nm", tag="nm")
                    yk_new = state.tile([128, 2 * W], f32, name="y_n", tag="y")
                    u_new = work.tile([128, W], f32, name="u_n", tag="u")

                    # ---- MM stream: w1 pass then fused pass ----
                    cnt = 0
                    total = 2 * NCH * NCH
                    first_mm = None
                    last_mm = None

                    def emit(mm):
                        nonlocal first_mm, last_mm
                        if first_mm is None:
                            first_mm = mm
                        last_mm = mm

                    hv0 = _hi_view(slot0(yk_cur))
                    for kc in range(NCH):
                        for mc in range(NCH):
                            cnt += 1
                            emit(nc.tensor.matmul(
                                zb[s][:, mc * BLOC:(mc + 1) * BLOC],
                                lhsT=w_tile(1, kc, mc),
                                rhs=hv0[:, kc * BLOC:(kc + 1) * BLOC],
                                start=False, stop=(cnt == total),
                                skip_group_check=True))
                    for kc in range(NCH):
                        frhs = fused_rhs(yk_cur, kc)
                        for mc in range(NCH):
                            cnt += 1
                            emit(nc.tensor.matmul(
                                zb[s][:, mc * BLOC:(mc + 1) * BLOC]
                                .unsqueeze(1).broadcast_to([128, 2, BLOC]),
                                lhsT=w_tile(0, kc, mc),
                                rhs=frhs,
                                start=False, stop=(cnt == total),
                                skip_group_check=True))
                    if _pe_chain[0] is not None:
                        add_dep_helper(first_mm.ins, _pe_chain[0].ins,
                                       sync=False, reason="PE ordering")
                    _pe_chain[0] = last_mm

                    # ---- merged epilogue ----
                    chain_act(nc.scalar.activation(
                        f_t[:], zb[s][:],
                        mybir.ActivationFunctionType.Tanh))

                    if recip == "fused2":
                        # pool WAR-guard lands on this early touch (runs in
                        # the tanh window) instead of stalling the mul
                        chain_dve(nc.vector.memset(yk_new[:, W:W + 1], 0.0))
                        # DVE: seed(f+p2) -> [nm fills d-wait] -> NR2 -> mul
                        # GPS computes d = f + p2 in parallel for the NR.
                        chain_gps(nc.gpsimd.tensor_add(
                            d_t[:], f_t[:], p2_cur[:]))
                        chain_dve(nc.vector._custom_dve(
                            seed_op, out=scr[:], in0=f_t[:], in1=p2_cur[:],
                            s0=-0.23549792, s1=2.0017324))
                        chain_dve(nc.vector.tensor_add(
                            nm_t[:], f_t[:], u_cur[:]))
                        chain_dve(nc.vector._custom_dve(
                            nr2_op, out=r_t[:], in0=d_t[:], in1=scr[:],
                            s0=2.0))
                    else:
                        chain_dve(nc.vector.tensor_add(
                            d_t[:], f_t[:], p2_cur[:]))
                        if recip == "accurate":
                            chain_dve(nc.vector.reciprocal(
                                out=r_t[:], in_=d_t[:]))
                        else:
                            chain_dve(nc.vector.reciprocal_approx_fast(
                                out=scr[:], in_=d_t[:]))
                            from concourse.dve_ops import RECIPROCAL_APPROX_NR
                            chain_dve(nc.vector._custom_dve(
                                RECIPROCAL_APPROX_NR, out=r_t[:], in0=d_t[:],
                                in1=scr[:], s0=2.0))
                        chain_gps(nc.gpsimd.tensor_add(
                            nm_t[:], f_t[:], u_cur[:]))
                    # split mul/sub into kc-halves: the next stream's head
                    # (w1 kc0-1) needs only slot0[:, 0:32], so it can start
                    # one DVE-op earlier; same for the fused pass vs slot1
                    H = W // 2
                    s0n = slot0(yk_new)
                    s1h = slot1_hi(yk_new)
                    chain_dve(nc.vector.tensor_mul(
                        s0n[:, 0:H], nm_t[:, 0:H], r_t[:, 0:H]))
                    chain_dve(nc.vector.tensor_mul(
                        s0n[:, H:], nm_t[:, H:], r_t[:, H:]))
                    chain_gps(nc.gpsimd.tensor_sub(
                        s1h[:, 0:H], s0n[:, 0:H], _hi_view(s0n)[:, 0:H]))
                    chain_gps(nc.gpsimd.tensor_sub(
                        s1h[:, H:], s0n[:, H:], _hi_view(s0n)[:, H:]))
                    if not last_unfold:
                        chain_gps(nc.gpsimd.tensor_mul(
                            u_new[:], s0n, rte_cur[:]))
                    elif t + 1 < t_run:
                        chain_gps(nc.gpsimd.tensor_mul(
                            u_new[:], s0n, rte_nxt[:]))

                    if gk + 1 < total_gk:
                        emit_junk()

                    if k == 2 and t + 1 < t_run:
                        rte_nxt = mk_rt_exp(t + 1)
                        p2_nxt = mk_p2(t + 1, rte_nxt)
                    yk_cur = yk_new
                    u_cur = u_new

                nc.sync.dma_start(yout_d[t][:, :], slot0(yk_cur))
                if t + 1 < t_run:
                    rte_cur, p2_cur = rte_nxt, p2_nxt

    t1 = _time.time()
    nc.compile()
    t2 = _time.time()
    if VERBOSE:
        print(f"[build] trace+schedule {t1-t0:.1f}s, bacc compile {t2-t1:.1f}s",
              flush=True)
    return nc


def _host_prep(i, delta_t, W_rec, W_in, b, A, tau, t_run):
    i = np.asarray(i, dtype=np.float32)
    delta_t = np.asarray(delta_t, dtype=np.float32)
    W_rec = np.asarray(W_rec, dtype=np.float32)
    W_in = np.asarray(W_in, dtype=np.float32)
    b = np.asarray(b, dtype=np.float32)
    A = np.asarray(A, dtype=np.float32)
    tau = np.asarray(tau, dtype=np.float32)

    def tiles_rec(m):
        return m.reshape(NCH, 128, NCH, 128).transpose(1, 0, 2, 3).reshape(128, -1)

    def tiles_in(m):
        return m.reshape(FCH, 128, NCH, 128).transpose(1, 0, 2, 3).reshape(128, -1)

    Wt = (W_rec * A[None, :]).T
    w_arrs = [np.ascontiguousarray(tiles_rec(x.astype(np.float32)).astype(x.dtype))
              for x in _bf16_split(Wt, 2)]
    win_arr = np.ascontiguousarray(tiles_in(W_in.T), dtype=np.float32)

    invtau = np.ascontiguousarray((1.0 / tau).reshape(NCH, 128).T, dtype=np.float32)
    bvec = np.ascontiguousarray(b.reshape(1, -1), dtype=np.float32)

    in_maps = []
    for c in range(NCORES):
        bsl = slice(c * BLOC, (c + 1) * BLOC)
        ii = i[bsl, :t_run]
        it = np.ascontiguousarray(
            ii.reshape(BLOC, t_run, FCH, 128).transpose(3, 2, 1, 0)
            .reshape(128, -1), dtype=np.float32)
        r = (KUNF / np.maximum(delta_t[bsl, :t_run], 1e-30)).T.reshape(1, -1)
        rtb = np.ascontiguousarray(
            np.broadcast_to(r, (128, r.shape[1])), dtype=np.float32)
        m = {"it": it, "rtb": rtb, "invtau": invtau, "bvec": bvec,
             "win": win_arr, "wrec0": w_arrs[0], "wrec1": w_arrs[1]}
        in_maps.append(m)
    return in_maps


def _host_unshard(results, A, t_run):
    A = np.asarray(A, dtype=np.float32)
    out = np.empty((B, t_run, N), dtype=np.float32)
    for c in range(NCORES):
        y = results[c]["yout"].reshape(t_run, 128, NCH, BLOC)
        xc = y.transpose(3, 0, 2, 1).reshape(BLOC, t_run, N)
        out[c * BLOC:(c + 1) * BLOC] = xc * A[None, None, :]
    return out


_BUILD_CACHE = {}


def _get_built(t_run, mm_mode):
    key = (t_run, mm_mode)
    if key not in _BUILD_CACHE:
        _BUILD_CACHE[key] = build(t_run, mm_mode)
    return _BUILD_CACHE[key]


def run(i, delta_t, W_rec, W_in, b, A, tau, t_run=T, mm_mode="split3",
        **rb_kwargs):
    nc = _get_built(t_run, mm_mode)
    in_maps = _host_prep(i, delta_t, W_rec, W_in, b, A, tau, t_run)
    res = run_bass_kernel_spmd(nc, in_maps, list(range(NCORES)), **rb_kwargs)
    out = _host_unshard(res.results, A, t_run)
    return out, res


MM_DTYPE = "split3"


def kernel(i, delta_t, W_rec, W_in, b, A, tau):
    out, _ = run(i, delta_t, W_rec, W_in, b, A, tau)
    return out


# revision 6
# speedup vs baseline: 1.1982x; 1.0213x over previous
"""Trainium2 Bass kernel for nn_AbstractLiquidRecurrent — v3 (merged epilogue).

Same math as v2 (split3, A folded, y'=(y*R+f)/(R+1/tau+f)) but:
  - G=1: ONE merged [128, 64] epilogue per unfold (one tanh, one d-add,
    one 2-op ~2ULP reciprocal, one mul, one slot1 sub, one u mul, one nm
    add) instead of two staggered 32-wide chains that serialized on DVE
    anyway.  Halves the per-op fixed-cost bill and the DVE serial span.
  - ONE z PSUM bank [128,64] per set; ONE inp preload ACT copy per unfold.
  - PE kept warm (HAM K=8/8) by long junk matmuls into the dead GEMM
    prologue bank during the PE idle window of each unfold.
  - State in one two-slot tile yk [128, 2, 64] f32: slot0 = y (w1 pass and
    fused pass read its hi16 by bitcast), slot1 carries y1 in hi16 halves.
  - MM stream per unfold: 16 w1 MMs (gated only on slot0) then 16 fused
    MMs (gated on slot1 written by one GPS sub).
"""

import time as _time

import numpy as np

import concourse.bass as bass
import concourse.tile as tile
from concourse.tile import add_dep_helper
from concourse import bacc, mybir
from concourse.bass_utils import run_bass_kernel_spmd

N = 512
F = 256
KUNF = 6
B, T = 128, 256
NCORES = 8
BLOC = B // NCORES          # 16 batch rows per core
NCH = N // 128              # 4 n-chunks
FCH = F // 128              # 2 f-chunks

f32 = mybir.dt.float32
bf16 = mybir.dt.bfloat16

VERBOSE = True
RECIP = "approx2"           # "approx2" | "fused2" (custom add+seed op, GPS d) | "accurate"
JUNK_MM = 0                 # PE-warming matmuls per unfold (measured: no help)


def _register_custom_recip_ops():
    """Two self-pinned custom DVE ops for the fused reciprocal path:
      ADD_RECIP_SEED_LIQ: out = seed+NR1 of 1/(in0+in1)   (~0.4% rel)
      RECIP_NR2_LIQ:      out = two Newton steps: in1 refined vs d=in0
    Together with a GPS-computed d they give ~2 ULP in 2 DVE ops with the
    d-add folded into the first op (saves one DVE op on the chain)."""
    import numpy as _np
    import concourse.dve_ops as dve_ops_mod
    from concourse.dve_ops import DveOp
    from concourse.dve_spec import (
        Spec, Src0, Src1, C0, C1, AluOp, Bin, lower,
        _has_src1 as has_src1,
    )
    from concourse.dve_uop import DveOpSpec

    if "ADD_RECIP_SEED_LIQ" in dve_ops_mod._SUB_OPCODE_FOR_NAME:
        return (dve_ops_mod.CUSTOM_DVE_OPS_LIQ)  # type: ignore[attr-defined]

    def _ref_seed(in0, in1, c0, c1, c2):
        d = (in0.astype(_np.float32) + in1.astype(_np.float32)).astype(_np.float32)
        nd = (~d.view(_np.int32)).view(_np.float32)
        y0 = (nd * _np.float32(c0)).astype(_np.float32)
        return (y0 * (_np.float32(c1) - d * y0)).astype(_np.float32)

    def _ref_nr2(in0, in1, c0, c1, c2):
        d = in0.astype(_np.float32)
        y = in1.astype(_np.float32)
        y2 = (y * (_np.float32(c0) - d * y)).astype(_np.float32)
        return (y2 * (_np.float32(c0) - d * y2)).astype(_np.float32)

    _d = Src0 + Src1
    _nd = Bin(AluOp.BITWISE_NOT, _d, _d)
    _s = _nd * C0
    seed_spec = Spec(body=_s * (C1 - _d * _s), reference=_ref_seed)
    _y2 = Src1 * (C0 - Src0 * Src1)
    nr2_spec = Spec(body=_y2 * (C0 - Src0 * _y2), reference=_ref_nr2)

    ops = []
    next_row = max(dve_ops_mod._SUB_OPCODE_FOR_NAME.values()) + 1
    for name, spec in (("ADD_RECIP_SEED_LIQ", seed_spec),
                       ("RECIP_NR2_LIQ", nr2_spec)):
        shas = {}
        for ver in ("v3", "v4"):
            try:
                compiled = DveOpSpec(name=name, opcode=next_row,
                                     uops=lower(spec, ver=ver),
                                     rd1_en=has_src1(spec))
                shas[ver] = compiled.sha(ver)
            except Exception:
                pass
        op = DveOp(name, spec, subdim=False, uops_sha=shas)
        dve_ops_mod.OPS.append(op)
        dve_ops_mod.CUSTOM_DVE_SPECS[name] = spec
        dve_ops_mod._SUB_OPCODE_FOR_NAME[name] = next_row
        next_row += 1
        ops.append(op)
    assert next_row <= 0x20, "custom DVE row overflow"
    dve_ops_mod.CUSTOM_DVE_OPS_LIQ = tuple(ops)  # type: ignore[attr-defined]
    return tuple(ops)


def _bf16_split(arr, terms):
    import ml_dtypes
    out = []
    rem = np.asarray(arr, dtype=np.float32).copy()
    for _ in range(terms):
        h = rem.astype(ml_dtypes.bfloat16)
        out.append(np.ascontiguousarray(h))
        rem = rem - h.astype(np.float32)
    return out


def _hi_view(ap):
    p, n = ap.shape
    return ap.bitcast(bf16).rearrange("p (n two) -> p n two", two=2)[:, :, 1]


def build(t_run=T, mm_mode="split3", recip=None, junk=None):
    assert mm_mode == "split3"
    recip = recip or RECIP
    junk = JUNK_MM if junk is None else junk
    if recip == "fused2":
        seed_op, nr2_op = _register_custom_recip_ops()
    t0 = _time.time()
    nc = bacc.Bacc("TRN2", target_bir_lowering=False, debug=False,
                   disable_frame_to_traceback=True)

    W = NCH * BLOC   # 64: merged state width
    TB = min(32, t_run)
    assert t_run % TB == 0
    NBLK = t_run // TB

    w_d = [nc.dram_tensor(f"wrec{j}", [128, NCH * NCH * 128], bf16,
                          kind="ExternalInput").ap() for j in range(2)]
    win_d = nc.dram_tensor("win", [128, FCH * NCH * 128], f32,
                           kind="ExternalInput").ap()
    it_d = nc.dram_tensor("it", [128, FCH * t_run * BLOC], f32,
                          kind="ExternalInput").ap()
    rtb_d = nc.dram_tensor("rtb", [128, t_run * BLOC], f32,
                           kind="ExternalInput").ap()
    invtau_d = nc.dram_tensor("invtau", [128, NCH], f32,
                              kind="ExternalInput").ap()
    bvec_d = nc.dram_tensor("bvec", [1, NCH * 128], f32,
                            kind="ExternalInput").ap()
    yout_d = nc.dram_tensor("yout", [t_run, 128, NCH * BLOC], f32,
                            kind="ExternalOutput").ap()

    with tile.TileContext(nc) as tc:
        import contextlib
        ctx = contextlib.ExitStack()
        with ctx:
            consts = ctx.enter_context(tc.tile_pool(name="consts", bufs=1))
            state = ctx.enter_context(tc.tile_pool(name="state", bufs=5))
            work = ctx.enter_context(tc.tile_pool(name="work", bufs=6))
            prep = ctx.enter_context(tc.tile_pool(name="prep", bufs=4))
            psum = ctx.enter_context(tc.tile_pool(name="psum", bufs=1, space="PSUM"))

            w_sb = []
            for j in range(2):
                wj = consts.tile([128, NCH * NCH * 128], bf16, name=f"w_sb{j}")
                nc.sync.dma_start(wj[:], w_d[j][:])
                w_sb.append(wj)
            win_sb = consts.tile([128, FCH * NCH * 128], f32, name="win_sb")
            nc.sync.dma_start(win_sb[:], win_d[:])
            it_sb = consts.tile([128, FCH * t_run * BLOC], f32, name="it_sb")
            nc.sync.dma_start(it_sb[:], it_d[:])
            rtb_sb = consts.tile([128, t_run * BLOC], f32, name="rtb_sb")
            nc.sync.dma_start(rtb_sb[:], rtb_d[:])
            invtau_sb = consts.tile([128, NCH], f32)
            nc.sync.dma_start(invtau_sb[:], invtau_d[:])
            bvec_sb = consts.tile([1, NCH * 128], f32)
            nc.sync.dma_start(bvec_sb[:], bvec_d[:])
            ones_sb = consts.tile([1, TB * BLOC], f32)
            nc.vector.memset(ones_sb[:], 1.0)
            junk1 = consts.tile([1, W], bf16)
            nc.vector.memset(junk1[:], 0.0)
            junk2 = consts.tile([1, 128], bf16)
            nc.vector.memset(junk2[:], 0.0)
            inp_sb = consts.tile([128, t_run * NCH * BLOC], f32, name="inp_sb")

            # PSUM: z banks (2 sets, merged [128, W]) + GEMM/junk banks
            zb = [psum.tile([128, W], f32, name=f"z{s}", tag=f"z{s}")
                  for s in range(2)]
            pg = [psum.tile([128, TB * BLOC], f32, name=f"pg{q}", tag=f"pg{q}")
                  for q in range(2)]

            _dve_chain = [None]
            _pe_chain = [None]
            _act_chain = [None]
            _gps_chain = [None]

            def _chain(slot, op, why):
                if slot[0] is not None:
                    add_dep_helper(op.ins, slot[0].ins, sync=False, reason=why)
                slot[0] = op
                return op

            def chain_dve(op):
                return _chain(_dve_chain, op, "DVE order")

            def chain_mm(op):
                return _chain(_pe_chain, op, "PE order")

            def chain_act(op):
                return _chain(_act_chain, op, "ACT order")

            def chain_gps(op):
                return _chain(_gps_chain, op, "GPS order")

            # ---- input projection GEMM: inp = i @ Win.T + b ----
            def win_tile(fc, mc):
                off = (fc * NCH + mc) * 128
                return win_sb[:, off:off + 128]

            for mc in range(NCH):
                for tb in range(NBLK):
                    bank = pg[(mc * NBLK + tb) % 2]
                    for fc in range(FCH):
                        base = fc * t_run * BLOC + tb * TB * BLOC
                        chain_mm(nc.tensor.matmul(
                            bank[:],
                            lhsT=win_tile(fc, mc),
                            rhs=it_sb[:, base:base + TB * BLOC],
                            start=(fc == 0), stop=False,
                            skip_group_check=True))
                    chain_mm(nc.tensor.matmul(
                        bank[:],
                        lhsT=bvec_sb[:, mc * 128:(mc + 1) * 128],
                        rhs=ones_sb[:],
                        start=False, stop=True,
                        skip_group_check=True))
                    dst = inp_sb[:].rearrange(
                        "p (t m b) -> p t m b", t=t_run, m=NCH)[
                        :, tb * TB:(tb + 1) * TB, mc, :]
                    chain_act(nc.scalar.activation(
                        dst, bank[:].rearrange("p (t b) -> p t b", t=TB),
                        mybir.ActivationFunctionType.Copy))

            # arm has_written bits of the z banks once
            for s in range(2):
                chain_mm(nc.tensor.matmul(
                    zb[s][:], lhsT=junk2[:], rhs=junk1[:],
                    start=True, stop=True))

            # ---- state: one two-slot tile ----
            def slot0(yk):
                return yk[:].rearrange("p (u b) -> p u b", u=2)[:, 0, :]

            def slot1_hi(yk):
                v = yk[:].bitcast(bf16).rearrange(
                    "p (u b two) -> p u b two", u=2, two=2)
                return v[:, 1, :, 1]

            def fused_rhs(yk, kc):
                v = yk[:].bitcast(bf16).rearrange(
                    "p (u b two) -> p u b two", u=2, two=2)[:, :, :, 1]
                return v[:, :, kc * BLOC:(kc + 1) * BLOC]

            yk_cur = state.tile([128, 2 * W], f32, name="y_init", tag="y")
            nc.vector.memset(yk_cur[:], 0.0)
            u_cur = work.tile([128, W], f32, name="u_init", tag="u")
            nc.vector.memset(u_cur[:], 0.0)

            def rt_slice(t):
                return rtb_sb[:, t * BLOC:(t + 1) * BLOC]

            def mk_rt_exp(t):
                rte = prep.tile([128, W], f32, tag="rte", name=f"rte{t}")
                chain_dve(nc.vector.tensor_copy(
                    rte[:].rearrange("p (m b) -> p m b", m=NCH),
                    rt_slice(t).unsqueeze(1).broadcast_to([128, NCH, BLOC])))
                return rte

            def mk_p2(t, rte):
                p2 = prep.tile([128, W], f32, tag="p2", name=f"p2_{t}")
                chain_dve(nc.vector.tensor_add(
                    p2[:].rearrange("p (m b) -> p m b", m=NCH),
                    rte[:].rearrange("p (m b) -> p m b", m=NCH),
                    invtau_sb[:, :].unsqueeze(2).broadcast_to([128, NCH, BLOC]),
                ))
                return p2

            def preload(bank, t2):
                src = inp_sb[:, t2 * W:(t2 + 1) * W]
                chain_act(nc.scalar.activation(
                    bank[:], src,
                    mybir.ActivationFunctionType.Copy))

            rte_cur = mk_rt_exp(0)
            p2_cur = mk_p2(0, rte_cur)
            preload(zb[0], 0)
            # unfold precision schedule: k0-k2 recompute z = W@y + inp in
            # full split3 (32 tiles); k3-k5 accumulate z += w0 @ hi16(dy)
            # (16 tiles) onto the PSUM-persistent z -- dy = y_k - y_{k-1} is
            # ~4% of |y| so a single bf16 pass holds the error budget.
            MODE = ("full", "full", "full", "single", "single", "single")

            def w_tile(j, kc, mc):
                off = (kc * NCH + mc) * 128
                return w_sb[j][:, off:off + 128]

            junk_rhs = w_sb[0][:, 0:512]

            def emit_junk():
                for _ in range(junk):
                    chain_mm(nc.tensor.matmul(
                        pg[0][:], lhsT=w_tile(0, 0, 0), rhs=junk_rhs,
                        start=True, stop=True, skip_group_check=True))

            rte_nxt = None
            p2_nxt = None
            dlt_cur = None
            total_gk = t_run * KUNF
            for t in range(t_run):
                for k in range(KUNF):
                    gk = t * KUNF + k
                    last_unfold = (k == KUNF - 1)
                    bank = zb[t % 2]
                    mode = MODE[k]
                    next_mode = MODE[(k + 1) % KUNF]

                    # re-preload inp for the k1/k2 full recomputes (ACT chain
                    # puts the copy after tanh(k-1)); t+1's bank at k==4
                    if k in (1, 2):
                        preload(bank, t)
                    if k == 4 and t + 1 < t_run:
                        preload(zb[(t + 1) % 2], t + 1)

                    # allocate all epilogue tiles up-front so their pool
                    # WAR-guard sems dispatch early and pre-satisfy
                    f_t = work.tile([128, W], f32, name="f", tag="f")
                    d_t = work.tile([128, W], f32, name="d", tag="d")
                    r_t = work.tile([128, W], f32, name="r", tag="r")
                    scr = work.tile([128, W], f32, name="rs", tag="rs")
                    nm_t = work.tile([128, W], f32, name="nm", tag="nm")
                    yk_new = state.tile([128, 2 * W], f32, name="y_n", tag="y")
                    u_new = work.tile([128, W], f32, name="u_n", tag="u")

                    # ---- MM stream ----
                    cnt = 0
                    total = 2 * NCH * NCH if mode == "full" else NCH * NCH
                    first_mm = None
                    last_mm = None

                    def emit(mm):
                        nonlocal first_mm, last_mm
                        if first_mm is None:
                            first_mm = mm
                        last_mm = mm

                    if mode == "full":
                        hv0 = _hi_view(slot0(yk_cur))
                        for kc in range(NCH):
                            for mc in range(NCH):
                                cnt += 1
                                emit(nc.tensor.matmul(
                                    bank[:, mc * BLOC:(mc + 1) * BLOC],
                                    lhsT=w_tile(1, kc, mc),
                                    rhs=hv0[:, kc * BLOC:(kc + 1) * BLOC],
                                    start=False, stop=(cnt == total),
                                    skip_group_check=True))
                        for kc in range(NCH):
                            frhs = fused_rhs(yk_cur, kc)
                            for mc in range(NCH):
                                cnt += 1
                                emit(nc.tensor.matmul(
                                    bank[:, mc * BLOC:(mc + 1) * BLOC]
                                    .unsqueeze(1).broadcast_to([128, 2, BLOC]),
                                    lhsT=w_tile(0, kc, mc),
                                    rhs=frhs,
                                    start=False, stop=(cnt == total),
                                    skip_group_check=True))
                    else:
                        hvD = _hi_view(dlt_cur[:])
                        for kc in range(NCH):
                            for mc in range(NCH):
                                cnt += 1
                                emit(nc.tensor.matmul(
                                    bank[:, mc * BLOC:(mc + 1) * BLOC],
                                    lhsT=w_tile(0, kc, mc),
                                    rhs=hvD[:, kc * BLOC:(kc + 1) * BLOC],
                                    start=False, stop=(cnt == total),
                                    skip_group_check=True))
                    if _pe_chain[0] is not None:
                        add_dep_helper(first_mm.ins, _pe_chain[0].ins,
                                       sync=False, reason="PE ordering")
                    _pe_chain[0] = last_mm

                    # ---- merged epilogue ----
                    chain_act(nc.scalar.activation(
                        f_t[:], bank[:],
                        mybir.ActivationFunctionType.Tanh))

                    if recip == "fused2":
                        # pool WAR-guard lands on this early touch (runs in
                        # the tanh window) instead of stalling the mul
                        chain_dve(nc.vector.memset(yk_new[:, W:W + 1], 0.0))
                        # DVE: seed(f+p2) -> [nm fills d-wait] -> NR2 -> mul
                        # GPS computes d = f + p2 in parallel for the NR.
                        chain_gps(nc.gpsimd.tensor_add(
                            d_t[:], f_t[:], p2_cur[:]))
                        chain_dve(nc.vector._custom_dve(
                            seed_op, out=scr[:], in0=f_t[:], in1=p2_cur[:],
                            s0=-0.23549792, s1=2.0017324))
                        chain_dve(nc.vector.tensor_add(
                            nm_t[:], f_t[:], u_cur[:]))
                        chain_dve(nc.vector._custom_dve(
                            nr2_op, out=r_t[:], in0=d_t[:], in1=scr[:],
                            s0=2.0))
                    else:
                        chain_dve(nc.vector.tensor_add(
                            d_t[:], f_t[:], p2_cur[:]))
                        if recip == "accurate":
                            chain_dve(nc.vector.reciprocal(
                                out=r_t[:], in_=d_t[:]))
                        else:
                            chain_dve(nc.vector.reciprocal_approx_fast(
                                out=scr[:], in_=d_t[:]))
                            from concourse.dve_ops import RECIPROCAL_APPROX_NR
                            chain_dve(nc.vector._custom_dve(
                                RECIPROCAL_APPROX_NR, out=r_t[:], in0=d_t[:],
                                in1=scr[:], s0=2.0))
                        chain_gps(nc.gpsimd.tensor_add(
                            nm_t[:], f_t[:], u_cur[:]))
                    # split mul (and the follow-up sub) into kc-halves: the
                    # next stream's kc0-1 MMs gate on the _a half only
                    H = W // 2
                    s0n = slot0(yk_new)
                    s1h = slot1_hi(yk_new)
                    s0o = slot0(yk_cur)
                    if next_mode == "single":
                        dlt_new = work.tile([128, W], f32, name="dd", tag="dd")
                        chain_dve(nc.vector.tensor_mul(
                            s0n[:, 0:H], nm_t[:, 0:H], r_t[:, 0:H]))
                        chain_dve(nc.vector.tensor_sub(
                            dlt_new[:, 0:H], s0n[:, 0:H], s0o[:, 0:H]))
                        chain_dve(nc.vector.tensor_mul(
                            s0n[:, H:], nm_t[:, H:], r_t[:, H:]))
                        chain_dve(nc.vector.tensor_sub(
                            dlt_new[:, H:], s0n[:, H:], s0o[:, H:]))
                        dlt_cur = dlt_new
                    else:
                        chain_dve(nc.vector.tensor_mul(
                            s0n[:, 0:H], nm_t[:, 0:H], r_t[:, 0:H]))
                        chain_dve(nc.vector.tensor_mul(
                            s0n[:, H:], nm_t[:, H:], r_t[:, H:]))
                        chain_gps(nc.gpsimd.tensor_sub(
                            s1h[:, 0:H], s0n[:, 0:H], _hi_view(s0n)[:, 0:H]))
                        chain_gps(nc.gpsimd.tensor_sub(
                            s1h[:, H:], s0n[:, H:], _hi_view(s0n)[:, H:]))
                    if not last_unfold:
                        chain_gps(nc.gpsimd.tensor_mul(
                            u_new[:], s0n, rte_cur[:]))
                    elif t + 1 < t_run:
                        chain_gps(nc.gpsimd.tensor_mul(
                            u_new[:], s0n, rte_nxt[:]))

                    if gk + 1 < total_gk:
                        emit_junk()

                    if k == 2 and t + 1 < t_run:
                        rte_nxt = mk_rt_exp(t + 1)
                        p2_nxt = mk_p2(t + 1, rte_nxt)
                    yk_cur = yk_new
                    u_cur = u_new

                nc.sync.dma_start(yout_d[t][:, :], slot0(yk_cur))
                if t + 1 < t_run:
                    rte_cur, p2_cur = rte_nxt, p2_nxt

    t1 = _time.time()
    nc.compile()
    t2 = _time.time()
    if VERBOSE:
        print(f"[build] trace+schedule {t1-t0:.1f}s, bacc compile {t2-t1:.1f}s",
              flush=True)
    return nc


def _host_prep(i, delta_t, W_rec, W_in, b, A, tau, t_run):
    i = np.asarray(i, dtype=np.float32)
    delta_t = np.asarray(delta_t, dtype=np.float32)
    W_rec = np.asarray(W_rec, dtype=np.float32)
    W_in = np.asarray(W_in, dtype=np.float32)
    b = np.asarray(b, dtype=np.float32)
    A = np.asarray(A, dtype=np.float32)
    tau = np.asarray(tau, dtype=np.float32)

    def tiles_rec(m):
        return m.reshape(NCH, 128, NCH, 128).transpose(1, 0, 2, 3).reshape(128, -1)

    def tiles_in(m):
        return m.reshape(FCH, 128, NCH, 128).transpose(1, 0, 2, 3).reshape(128, -1)

    Wt = (W_rec * A[None, :]).T
    w_arrs = [np.ascontiguousarray(tiles_rec(x.astype(np.float32)).astype(x.dtype))
              for x in _bf16_split(Wt, 2)]
    win_arr = np.ascontiguousarray(tiles_in(W_in.T), dtype=np.float32)

    invtau = np.ascontiguousarray((1.0 / tau).reshape(NCH, 128).T, dtype=np.float32)
    bvec = np.ascontiguousarray(b.reshape(1, -1), dtype=np.float32)

    in_maps = []
    for c in range(NCORES):
        bsl = slice(c * BLOC, (c + 1) * BLOC)
        ii = i[bsl, :t_run]
        it = np.ascontiguousarray(
            ii.reshape(BLOC, t_run, FCH, 128).transpose(3, 2, 1, 0)
            .reshape(128, -1), dtype=np.float32)
        r = (KUNF / np.maximum(delta_t[bsl, :t_run], 1e-30)).T.reshape(1, -1)
        rtb = np.ascontiguousarray(
            np.broadcast_to(r, (128, r.shape[1])), dtype=np.float32)
        m = {"it": it, "rtb": rtb, "invtau": invtau, "bvec": bvec,
             "win": win_arr, "wrec0": w_arrs[0], "wrec1": w_arrs[1]}
        in_maps.append(m)
    return in_maps


def _host_unshard(results, A, t_run):
    A = np.asarray(A, dtype=np.float32)
    out = np.empty((B, t_run, N), dtype=np.float32)
    for c in range(NCORES):
        y = results[c]["yout"].reshape(t_run, 128, NCH, BLOC)
        xc = y.transpose(3, 0, 2, 1).reshape(BLOC, t_run, N)
        out[c * BLOC:(c + 1) * BLOC] = xc * A[None, None, :]
    return out


_BUILD_CACHE = {}


def _get_built(t_run, mm_mode):
    key = (t_run, mm_mode)
    if key not in _BUILD_CACHE:
        _BUILD_CACHE[key] = build(t_run, mm_mode)
    return _BUILD_CACHE[key]


def run(i, delta_t, W_rec, W_in, b, A, tau, t_run=T, mm_mode="split3",
        **rb_kwargs):
    nc = _get_built(t_run, mm_mode)
    in_maps = _host_prep(i, delta_t, W_rec, W_in, b, A, tau, t_run)
    res = run_bass_kernel_spmd(nc, in_maps, list(range(NCORES)), **rb_kwargs)
    out = _host_unshard(res.results, A, t_run)
    return out, res


MM_DTYPE = "split3"


def kernel(i, delta_t, W_rec, W_in, b, A, tau):
    out, _ = run(i, delta_t, W_rec, W_in, b, A, tau)
    return out


# revision 7
# speedup vs baseline: 1.2262x; 1.0233x over previous
"""Trainium2 Bass kernel for nn_AbstractLiquidRecurrent — v3 (merged epilogue).

Same math as v2 (split3, A folded, y'=(y*R+f)/(R+1/tau+f)) but:
  - G=1: ONE merged [128, 64] epilogue per unfold (one tanh, one d-add,
    one 2-op ~2ULP reciprocal, one mul, one slot1 sub, one u mul, one nm
    add) instead of two staggered 32-wide chains that serialized on DVE
    anyway.  Halves the per-op fixed-cost bill and the DVE serial span.
  - ONE z PSUM bank [128,64] per set; ONE inp preload ACT copy per unfold.
  - PE kept warm (HAM K=8/8) by long junk matmuls into the dead GEMM
    prologue bank during the PE idle window of each unfold.
  - State in one two-slot tile yk [128, 2, 64] f32: slot0 = y (w1 pass and
    fused pass read its hi16 by bitcast), slot1 carries y1 in hi16 halves.
  - MM stream per unfold: 16 w1 MMs (gated only on slot0) then 16 fused
    MMs (gated on slot1 written by one GPS sub).
"""

import time as _time

import numpy as np

import concourse.bass as bass
import concourse.tile as tile
from concourse.tile import add_dep_helper
from concourse import bacc, mybir
from concourse.bass_utils import run_bass_kernel_spmd

N = 512
F = 256
KUNF = 6
B, T = 128, 256
NCORES = 8
BLOC = B // NCORES          # 16 batch rows per core
NCH = N // 128              # 4 n-chunks
FCH = F // 128              # 2 f-chunks

f32 = mybir.dt.float32
bf16 = mybir.dt.bfloat16

VERBOSE = True
RECIP = "approx2"           # "approx2" | "fused2" (custom add+seed op, GPS d) | "accurate"
JUNK_MM = 0                 # PE-warming matmuls per unfold (measured: no help)


def _register_custom_recip_ops():
    """Two self-pinned custom DVE ops for the fused reciprocal path:
      ADD_RECIP_SEED_LIQ: out = seed+NR1 of 1/(in0+in1)   (~0.4% rel)
      RECIP_NR2_LIQ:      out = two Newton steps: in1 refined vs d=in0
    Together with a GPS-computed d they give ~2 ULP in 2 DVE ops with the
    d-add folded into the first op (saves one DVE op on the chain)."""
    import numpy as _np
    import concourse.dve_ops as dve_ops_mod
    from concourse.dve_ops import DveOp
    from concourse.dve_spec import (
        Spec, Src0, Src1, C0, C1, AluOp, Bin, lower,
        _has_src1 as has_src1,
    )
    from concourse.dve_uop import DveOpSpec

    if "ADD_RECIP_SEED_LIQ" in dve_ops_mod._SUB_OPCODE_FOR_NAME:
        return (dve_ops_mod.CUSTOM_DVE_OPS_LIQ)  # type: ignore[attr-defined]

    def _ref_seed(in0, in1, c0, c1, c2):
        d = (in0.astype(_np.float32) + in1.astype(_np.float32)).astype(_np.float32)
        nd = (~d.view(_np.int32)).view(_np.float32)
        y0 = (nd * _np.float32(c0)).astype(_np.float32)
        return (y0 * (_np.float32(c1) - d * y0)).astype(_np.float32)

    def _ref_nr2(in0, in1, c0, c1, c2):
        d = in0.astype(_np.float32)
        y = in1.astype(_np.float32)
        y2 = (y * (_np.float32(c0) - d * y)).astype(_np.float32)
        return (y2 * (_np.float32(c0) - d * y2)).astype(_np.float32)

    _d = Src0 + Src1
    _nd = Bin(AluOp.BITWISE_NOT, _d, _d)
    _s = _nd * C0
    seed_spec = Spec(body=_s * (C1 - _d * _s), reference=_ref_seed)
    _y2 = Src1 * (C0 - Src0 * Src1)
    nr2_spec = Spec(body=_y2 * (C0 - Src0 * _y2), reference=_ref_nr2)

    ops = []
    next_row = max(dve_ops_mod._SUB_OPCODE_FOR_NAME.values()) + 1
    for name, spec in (("ADD_RECIP_SEED_LIQ", seed_spec),
                       ("RECIP_NR2_LIQ", nr2_spec)):
        shas = {}
        for ver in ("v3", "v4"):
            try:
                compiled = DveOpSpec(name=name, opcode=next_row,
                                     uops=lower(spec, ver=ver),
                                     rd1_en=has_src1(spec))
                shas[ver] = compiled.sha(ver)
            except Exception:
                pass
        op = DveOp(name, spec, subdim=False, uops_sha=shas)
        dve_ops_mod.OPS.append(op)
        dve_ops_mod.CUSTOM_DVE_SPECS[name] = spec
        dve_ops_mod._SUB_OPCODE_FOR_NAME[name] = next_row
        next_row += 1
        ops.append(op)
    assert next_row <= 0x20, "custom DVE row overflow"
    dve_ops_mod.CUSTOM_DVE_OPS_LIQ = tuple(ops)  # type: ignore[attr-defined]
    return tuple(ops)


def _bf16_split(arr, terms):
    import ml_dtypes
    out = []
    rem = np.asarray(arr, dtype=np.float32).copy()
    for _ in range(terms):
        h = rem.astype(ml_dtypes.bfloat16)
        out.append(np.ascontiguousarray(h))
        rem = rem - h.astype(np.float32)
    return out


def _hi_view(ap):
    p, n = ap.shape
    return ap.bitcast(bf16).rearrange("p (n two) -> p n two", two=2)[:, :, 1]


def build(t_run=T, mm_mode="split3", recip=None, junk=None):
    assert mm_mode == "split3"
    recip = recip or RECIP
    junk = JUNK_MM if junk is None else junk
    if recip == "fused2":
        seed_op, nr2_op = _register_custom_recip_ops()
    t0 = _time.time()
    nc = bacc.Bacc("TRN2", target_bir_lowering=False, debug=False,
                   disable_frame_to_traceback=True)

    W = NCH * BLOC   # 64: merged state width
    TB = min(32, t_run)
    assert t_run % TB == 0
    NBLK = t_run // TB

    w_d = [nc.dram_tensor(f"wrec{j}", [128, NCH * NCH * 128], bf16,
                          kind="ExternalInput").ap() for j in range(2)]
    win_d = nc.dram_tensor("win", [128, FCH * NCH * 128], f32,
                           kind="ExternalInput").ap()
    it_d = nc.dram_tensor("it", [128, FCH * t_run * BLOC], f32,
                          kind="ExternalInput").ap()
    rtb_d = nc.dram_tensor("rtb", [128, t_run * BLOC], f32,
                           kind="ExternalInput").ap()
    invtau_d = nc.dram_tensor("invtau", [128, NCH], f32,
                              kind="ExternalInput").ap()
    bvec_d = nc.dram_tensor("bvec", [1, NCH * 128], f32,
                            kind="ExternalInput").ap()
    yout_d = nc.dram_tensor("yout", [t_run, 128, NCH * BLOC], f32,
                            kind="ExternalOutput").ap()

    with tile.TileContext(nc) as tc:
        import contextlib
        ctx = contextlib.ExitStack()
        with ctx:
            consts = ctx.enter_context(tc.tile_pool(name="consts", bufs=1))
            # state bufs=8: the output DMA reads the k5 y-tile; a deeper
            # rotation keeps its WAR guard pre-satisfied at the t boundary
            state = ctx.enter_context(tc.tile_pool(name="state", bufs=8))
            work = ctx.enter_context(tc.tile_pool(name="work", bufs=6))
            prep = ctx.enter_context(tc.tile_pool(name="prep", bufs=4))
            psum = ctx.enter_context(tc.tile_pool(name="psum", bufs=1, space="PSUM"))

            w_sb = []
            for j in range(2):
                wj = consts.tile([128, NCH * NCH * 128], bf16, name=f"w_sb{j}")
                nc.sync.dma_start(wj[:], w_d[j][:])
                w_sb.append(wj)
            win_sb = consts.tile([128, FCH * NCH * 128], f32, name="win_sb")
            nc.sync.dma_start(win_sb[:], win_d[:])
            it_sb = consts.tile([128, FCH * t_run * BLOC], f32, name="it_sb")
            nc.sync.dma_start(it_sb[:], it_d[:])
            rtb_sb = consts.tile([128, t_run * BLOC], f32, name="rtb_sb")
            nc.sync.dma_start(rtb_sb[:], rtb_d[:])
            invtau_sb = consts.tile([128, NCH], f32)
            nc.sync.dma_start(invtau_sb[:], invtau_d[:])
            bvec_sb = consts.tile([1, NCH * 128], f32)
            nc.sync.dma_start(bvec_sb[:], bvec_d[:])
            ones_sb = consts.tile([1, TB * BLOC], f32)
            nc.vector.memset(ones_sb[:], 1.0)
            junk1 = consts.tile([1, W], bf16)
            nc.vector.memset(junk1[:], 0.0)
            junk2 = consts.tile([1, 128], bf16)
            nc.vector.memset(junk2[:], 0.0)
            inp_sb = consts.tile([128, t_run * NCH * BLOC], f32, name="inp_sb")

            # PSUM: z banks (2 sets, merged [128, W]) + GEMM/junk banks
            zb = [psum.tile([128, W], f32, name=f"z{s}", tag=f"z{s}")
                  for s in range(2)]
            pg = [psum.tile([128, TB * BLOC], f32, name=f"pg{q}", tag=f"pg{q}")
                  for q in range(2)]

            _dve_chain = [None]
            _pe_chain = [None]
            _act_chain = [None]
            _gps_chain = [None]

            def _chain(slot, op, why):
                if slot[0] is not None:
                    add_dep_helper(op.ins, slot[0].ins, sync=False, reason=why)
                slot[0] = op
                return op

            def chain_dve(op):
                return _chain(_dve_chain, op, "DVE order")

            def chain_mm(op):
                return _chain(_pe_chain, op, "PE order")

            def chain_act(op):
                return _chain(_act_chain, op, "ACT order")

            def chain_gps(op):
                return _chain(_gps_chain, op, "GPS order")

            # ---- input projection GEMM: inp = i @ Win.T + b ----
            def win_tile(fc, mc):
                off = (fc * NCH + mc) * 128
                return win_sb[:, off:off + 128]

            for mc in range(NCH):
                for tb in range(NBLK):
                    bank = pg[(mc * NBLK + tb) % 2]
                    for fc in range(FCH):
                        base = fc * t_run * BLOC + tb * TB * BLOC
                        chain_mm(nc.tensor.matmul(
                            bank[:],
                            lhsT=win_tile(fc, mc),
                            rhs=it_sb[:, base:base + TB * BLOC],
                            start=(fc == 0), stop=False,
                            skip_group_check=True))
                    chain_mm(nc.tensor.matmul(
                        bank[:],
                        lhsT=bvec_sb[:, mc * 128:(mc + 1) * 128],
                        rhs=ones_sb[:],
                        start=False, stop=True,
                        skip_group_check=True))
                    dst = inp_sb[:].rearrange(
                        "p (t m b) -> p t m b", t=t_run, m=NCH)[
                        :, tb * TB:(tb + 1) * TB, mc, :]
                    chain_act(nc.scalar.activation(
                        dst, bank[:].rearrange("p (t b) -> p t b", t=TB),
                        mybir.ActivationFunctionType.Copy))

            # arm has_written bits of the z banks once
            for s in range(2):
                chain_mm(nc.tensor.matmul(
                    zb[s][:], lhsT=junk2[:], rhs=junk1[:],
                    start=True, stop=True))

            # ---- state: one two-slot tile ----
            def slot0(yk):
                return yk[:].rearrange("p (u b) -> p u b", u=2)[:, 0, :]

            def slot1_hi(yk):
                v = yk[:].bitcast(bf16).rearrange(
                    "p (u b two) -> p u b two", u=2, two=2)
                return v[:, 1, :, 1]

            def fused_rhs(yk, kc):
                v = yk[:].bitcast(bf16).rearrange(
                    "p (u b two) -> p u b two", u=2, two=2)[:, :, :, 1]
                return v[:, :, kc * BLOC:(kc + 1) * BLOC]

            yk_cur = state.tile([128, 2 * W], f32, name="y_init", tag="y")
            nc.vector.memset(yk_cur[:], 0.0)
            u_cur = work.tile([128, W], f32, name="u_init", tag="u")
            nc.vector.memset(u_cur[:], 0.0)

            def rt_slice(t):
                return rtb_sb[:, t * BLOC:(t + 1) * BLOC]

            def mk_rt_exp(t):
                rte = prep.tile([128, W], f32, tag="rte", name=f"rte{t}")
                chain_dve(nc.vector.tensor_copy(
                    rte[:].rearrange("p (m b) -> p m b", m=NCH),
                    rt_slice(t).unsqueeze(1).broadcast_to([128, NCH, BLOC])))
                return rte

            def mk_p2(t, rte):
                p2 = prep.tile([128, W], f32, tag="p2", name=f"p2_{t}")
                chain_dve(nc.vector.tensor_add(
                    p2[:].rearrange("p (m b) -> p m b", m=NCH),
                    rte[:].rearrange("p (m b) -> p m b", m=NCH),
                    invtau_sb[:, :].unsqueeze(2).broadcast_to([128, NCH, BLOC]),
                ))
                return p2

            def preload(bank, t2):
                src = inp_sb[:, t2 * W:(t2 + 1) * W]
                chain_act(nc.scalar.activation(
                    bank[:], src,
                    mybir.ActivationFunctionType.Copy))

            rte_cur = mk_rt_exp(0)
            p2_cur = mk_p2(0, rte_cur)
            preload(zb[0], 0)
            # unfold precision schedule: k0-k2 recompute z = W@y + inp in
            # full split3 (32 tiles); k3-k5 accumulate z += w0 @ hi16(dy)
            # (16 tiles) onto the PSUM-persistent z -- dy = y_k - y_{k-1} is
            # ~4% of |y| so a single bf16 pass holds the error budget.
            MODE = ("full", "full", "full", "single", "single", "single")

            def w_tile(j, kc, mc):
                off = (kc * NCH + mc) * 128
                return w_sb[j][:, off:off + 128]

            junk_rhs = w_sb[0][:, 0:512]

            def emit_junk():
                for _ in range(junk):
                    chain_mm(nc.tensor.matmul(
                        pg[0][:], lhsT=w_tile(0, 0, 0), rhs=junk_rhs,
                        start=True, stop=True, skip_group_check=True))

            rte_nxt = None
            p2_nxt = None
            dlt_cur = None
            total_gk = t_run * KUNF
            for t in range(t_run):
                for k in range(KUNF):
                    gk = t * KUNF + k
                    last_unfold = (k == KUNF - 1)
                    bank = zb[t % 2]
                    mode = MODE[k]
                    next_mode = MODE[(k + 1) % KUNF]

                    # re-preload inp for the k1/k2 full recomputes (ACT chain
                    # puts the copy after tanh(k-1)); t+1's bank at k==4
                    if k in (1, 2):
                        preload(bank, t)
                    if k == 4 and t + 1 < t_run:
                        preload(zb[(t + 1) % 2], t + 1)

                    # allocate all epilogue tiles up-front so their pool
                    # WAR-guard sems dispatch early and pre-satisfy
                    f_t = work.tile([128, W], f32, name="f", tag="f")
                    d_t = work.tile([128, W], f32, name="d", tag="d")
                    r_t = work.tile([128, W], f32, name="r", tag="r")
                    scr = work.tile([128, W], f32, name="rs", tag="rs")
                    nm_t = work.tile([128, W], f32, name="nm", tag="nm")
                    yk_new = state.tile([128, 2 * W], f32, name="y_n", tag="y")
                    u_new = work.tile([128, W], f32, name="u_n", tag="u")

                    # ---- MM stream ----
                    cnt = 0
                    total = 2 * NCH * NCH if mode == "full" else NCH * NCH
                    first_mm = None
                    last_mm = None

                    def emit(mm):
                        nonlocal first_mm, last_mm
                        if first_mm is None:
                            first_mm = mm
                        last_mm = mm

                    if mode == "full":
                        hv0 = _hi_view(slot0(yk_cur))
                        for kc in range(NCH):
                            for mc in range(NCH):
                                cnt += 1
                                emit(nc.tensor.matmul(
                                    bank[:, mc * BLOC:(mc + 1) * BLOC],
                                    lhsT=w_tile(1, kc, mc),
                                    rhs=hv0[:, kc * BLOC:(kc + 1) * BLOC],
                                    start=False, stop=(cnt == total),
                                    skip_group_check=True))
                        for kc in range(NCH):
                            frhs = fused_rhs(yk_cur, kc)
                            for mc in range(NCH):
                                cnt += 1
                                emit(nc.tensor.matmul(
                                    bank[:, mc * BLOC:(mc + 1) * BLOC]
                                    .unsqueeze(1).broadcast_to([128, 2, BLOC]),
                                    lhsT=w_tile(0, kc, mc),
                                    rhs=frhs,
                                    start=False, stop=(cnt == total),
                                    skip_group_check=True))
                    else:
                        hvD = _hi_view(dlt_cur[:])
                        for kc in range(NCH):
                            for mc in range(NCH):
                                cnt += 1
                                emit(nc.tensor.matmul(
                                    bank[:, mc * BLOC:(mc + 1) * BLOC],
                                    lhsT=w_tile(0, kc, mc),
                                    rhs=hvD[:, kc * BLOC:(kc + 1) * BLOC],
                                    start=False, stop=(cnt == total),
                                    skip_group_check=True))
                    if _pe_chain[0] is not None:
                        add_dep_helper(first_mm.ins, _pe_chain[0].ins,
                                       sync=False, reason="PE ordering")
                    _pe_chain[0] = last_mm

                    # ---- merged epilogue ----
                    chain_act(nc.scalar.activation(
                        f_t[:], bank[:],
                        mybir.ActivationFunctionType.Tanh))

                    if recip == "fused2":
                        # pool WAR-guard lands on this early touch (runs in
                        # the tanh window) instead of stalling the mul
                        chain_dve(nc.vector.memset(yk_new[:, W:W + 1], 0.0))
                        # DVE: seed(f+p2) -> [nm fills d-wait] -> NR2 -> mul
                        # GPS computes d = f + p2 in parallel for the NR.
                        chain_gps(nc.gpsimd.tensor_add(
                            d_t[:], f_t[:], p2_cur[:]))
                        chain_dve(nc.vector._custom_dve(
                            seed_op, out=scr[:], in0=f_t[:], in1=p2_cur[:],
                            s0=-0.23549792, s1=2.0017324))
                        chain_dve(nc.vector.tensor_add(
                            nm_t[:], f_t[:], u_cur[:]))
                        chain_dve(nc.vector._custom_dve(
                            nr2_op, out=r_t[:], in0=d_t[:], in1=scr[:],
                            s0=2.0))
                    else:
                        chain_dve(nc.vector.tensor_add(
                            d_t[:], f_t[:], p2_cur[:]))
                        if recip == "accurate":
                            chain_dve(nc.vector.reciprocal(
                                out=r_t[:], in_=d_t[:]))
                        else:
                            chain_dve(nc.vector.reciprocal_approx_fast(
                                out=scr[:], in_=d_t[:]))
                            from concourse.dve_ops import RECIPROCAL_APPROX_NR
                            chain_dve(nc.vector._custom_dve(
                                RECIPROCAL_APPROX_NR, out=r_t[:], in0=d_t[:],
                                in1=scr[:], s0=2.0))
                        chain_gps(nc.gpsimd.tensor_add(
                            nm_t[:], f_t[:], u_cur[:]))
                    # split mul (and the follow-up sub) into kc-halves: the
                    # next stream's kc0-1 MMs gate on the _a half only
                    H = W // 2
                    s0n = slot0(yk_new)
                    s1h = slot1_hi(yk_new)
                    s0o = slot0(yk_cur)
                    if next_mode == "single":
                        dlt_new = work.tile([128, W], f32, name="dd", tag="dd")
                        chain_dve(nc.vector.tensor_mul(
                            s0n[:, 0:H], nm_t[:, 0:H], r_t[:, 0:H]))
                        chain_dve(nc.vector.tensor_sub(
                            dlt_new[:, 0:H], s0n[:, 0:H], s0o[:, 0:H]))
                        chain_dve(nc.vector.tensor_mul(
                            s0n[:, H:], nm_t[:, H:], r_t[:, H:]))
                        chain_dve(nc.vector.tensor_sub(
                            dlt_new[:, H:], s0n[:, H:], s0o[:, H:]))
                        dlt_cur = dlt_new
                    else:
                        chain_dve(nc.vector.tensor_mul(
                            s0n[:, 0:H], nm_t[:, 0:H], r_t[:, 0:H]))
                        chain_dve(nc.vector.tensor_mul(
                            s0n[:, H:], nm_t[:, H:], r_t[:, H:]))
                        chain_gps(nc.gpsimd.tensor_sub(
                            s1h[:, 0:H], s0n[:, 0:H], _hi_view(s0n)[:, 0:H]))
                        chain_gps(nc.gpsimd.tensor_sub(
                            s1h[:, H:], s0n[:, H:], _hi_view(s0n)[:, H:]))
                    if not last_unfold:
                        chain_gps(nc.gpsimd.tensor_mul(
                            u_new[:], s0n, rte_cur[:]))
                    elif t + 1 < t_run:
                        chain_gps(nc.gpsimd.tensor_mul(
                            u_new[:], s0n, rte_nxt[:]))

                    if gk + 1 < total_gk:
                        emit_junk()

                    if k == 2 and t + 1 < t_run:
                        rte_nxt = mk_rt_exp(t + 1)
                        p2_nxt = mk_p2(t + 1, rte_nxt)
                    yk_cur = yk_new
                    u_cur = u_new

                nc.sync.dma_start(yout_d[t][:, :], slot0(yk_cur))
                if t + 1 < t_run:
                    rte_cur, p2_cur = rte_nxt, p2_nxt

    t1 = _time.time()
    nc.compile()
    t2 = _time.time()
    if VERBOSE:
        print(f"[build] trace+schedule {t1-t0:.1f}s, bacc compile {t2-t1:.1f}s",
              flush=True)
    return nc


def _host_prep(i, delta_t, W_rec, W_in, b, A, tau, t_run):
    i = np.asarray(i, dtype=np.float32)
    delta_t = np.asarray(delta_t, dtype=np.float32)
    W_rec = np.asarray(W_rec, dtype=np.float32)
    W_in = np.asarray(W_in, dtype=np.float32)
    b = np.asarray(b, dtype=np.float32)
    A = np.asarray(A, dtype=np.float32)
    tau = np.asarray(tau, dtype=np.float32)

    def tiles_rec(m):
        return m.reshape(NCH, 128, NCH, 128).transpose(1, 0, 2, 3).reshape(128, -1)

    def tiles_in(m):
        return m.reshape(FCH, 128, NCH, 128).transpose(1, 0, 2, 3).reshape(128, -1)

    Wt = (W_rec * A[None, :]).T
    w_arrs = [np.ascontiguousarray(tiles_rec(x.astype(np.float32)).astype(x.dtype))
              for x in _bf16_split(Wt, 2)]
    win_arr = np.ascontiguousarray(tiles_in(W_in.T), dtype=np.float32)

    invtau = np.ascontiguousarray((1.0 / tau).reshape(NCH, 128).T, dtype=np.float32)
    bvec = np.ascontiguousarray(b.reshape(1, -1), dtype=np.float32)

    in_maps = []
    for c in range(NCORES):
        bsl = slice(c * BLOC, (c + 1) * BLOC)
        ii = i[bsl, :t_run]
        it = np.ascontiguousarray(
            ii.reshape(BLOC, t_run, FCH, 128).transpose(3, 2, 1, 0)
            .reshape(128, -1), dtype=np.float32)
        r = (KUNF / np.maximum(delta_t[bsl, :t_run], 1e-30)).T.reshape(1, -1)
        rtb = np.ascontiguousarray(
            np.broadcast_to(r, (128, r.shape[1])), dtype=np.float32)
        m = {"it": it, "rtb": rtb, "invtau": invtau, "bvec": bvec,
             "win": win_arr, "wrec0": w_arrs[0], "wrec1": w_arrs[1]}
        in_maps.append(m)
    return in_maps


def _host_unshard(results, A, t_run):
    A = np.asarray(A, dtype=np.float32)
    out = np.empty((B, t_run, N), dtype=np.float32)
    for c in range(NCORES):
        y = results[c]["yout"].reshape(t_run, 128, NCH, BLOC)
        xc = y.transpose(3, 0, 2, 1).reshape(BLOC, t_run, N)
        out[c * BLOC:(c + 1) * BLOC] = xc * A[None, None, :]
    return out


_BUILD_CACHE = {}


def _get_built(t_run, mm_mode):
    key = (t_run, mm_mode)
    if key not in _BUILD_CACHE:
        _BUILD_CACHE[key] = build(t_run, mm_mode)
    return _BUILD_CACHE[key]


def run(i, delta_t, W_rec, W_in, b, A, tau, t_run=T, mm_mode="split3",
        **rb_kwargs):
    nc = _get_built(t_run, mm_mode)
    in_maps = _host_prep(i, delta_t, W_rec, W_in, b, A, tau, t_run)
    res = run_bass_kernel_spmd(nc, in_maps, list(range(NCORES)), **rb_kwargs)
    out = _host_unshard(res.results, A, t_run)
    return out, res


MM_DTYPE = "split3"


def kernel(i, delta_t, W_rec, W_in, b, A, tau):
    out, _ = run(i, delta_t, W_rec, W_in, b, A, tau)
    return out


# revision 8
# speedup vs baseline: 1.2342x; 1.0065x over previous
"""Trainium2 Bass kernel for nn_AbstractLiquidRecurrent — v3 (merged epilogue).

Same math as v2 (split3, A folded, y'=(y*R+f)/(R+1/tau+f)) but:
  - G=1: ONE merged [128, 64] epilogue per unfold (one tanh, one d-add,
    one 2-op ~2ULP reciprocal, one mul, one slot1 sub, one u mul, one nm
    add) instead of two staggered 32-wide chains that serialized on DVE
    anyway.  Halves the per-op fixed-cost bill and the DVE serial span.
  - ONE z PSUM bank [128,64] per set; ONE inp preload ACT copy per unfold.
  - PE kept warm (HAM K=8/8) by long junk matmuls into the dead GEMM
    prologue bank during the PE idle window of each unfold.
  - State in one two-slot tile yk [128, 2, 64] f32: slot0 = y (w1 pass and
    fused pass read its hi16 by bitcast), slot1 carries y1 in hi16 halves.
  - MM stream per unfold: 16 w1 MMs (gated only on slot0) then 16 fused
    MMs (gated on slot1 written by one GPS sub).
"""

import time as _time

import numpy as np

import concourse.bass as bass
import concourse.tile as tile
from concourse.tile import add_dep_helper
from concourse import bacc, mybir
from concourse.bass_utils import run_bass_kernel_spmd

N = 512
F = 256
KUNF = 6
B, T = 128, 256
NCORES = 8
BLOC = B // NCORES          # 16 batch rows per core
NCH = N // 128              # 4 n-chunks
FCH = F // 128              # 2 f-chunks

f32 = mybir.dt.float32
bf16 = mybir.dt.bfloat16

VERBOSE = True
RECIP = "approx2"           # "approx2" | "fused2" (custom add+seed op, GPS d) | "accurate"
JUNK_MM = 0                 # PE-warming matmuls per unfold (measured: no help)


def _register_custom_recip_ops():
    """Two self-pinned custom DVE ops for the fused reciprocal path:
      ADD_RECIP_SEED_LIQ: out = seed+NR1 of 1/(in0+in1)   (~0.4% rel)
      RECIP_NR2_LIQ:      out = two Newton steps: in1 refined vs d=in0
    Together with a GPS-computed d they give ~2 ULP in 2 DVE ops with the
    d-add folded into the first op (saves one DVE op on the chain)."""
    import numpy as _np
    import concourse.dve_ops as dve_ops_mod
    from concourse.dve_ops import DveOp
    from concourse.dve_spec import (
        Spec, Src0, Src1, C0, C1, AluOp, Bin, lower,
        _has_src1 as has_src1,
    )
    from concourse.dve_uop import DveOpSpec

    if "ADD_RECIP_SEED_LIQ" in dve_ops_mod._SUB_OPCODE_FOR_NAME:
        return (dve_ops_mod.CUSTOM_DVE_OPS_LIQ)  # type: ignore[attr-defined]

    def _ref_seed(in0, in1, c0, c1, c2):
        d = (in0.astype(_np.float32) + in1.astype(_np.float32)).astype(_np.float32)
        nd = (~d.view(_np.int32)).view(_np.float32)
        y0 = (nd * _np.float32(c0)).astype(_np.float32)
        return (y0 * (_np.float32(c1) - d * y0)).astype(_np.float32)

    def _ref_nr2(in0, in1, c0, c1, c2):
        d = in0.astype(_np.float32)
        y = in1.astype(_np.float32)
        y2 = (y * (_np.float32(c0) - d * y)).astype(_np.float32)
        return (y2 * (_np.float32(c0) - d * y2)).astype(_np.float32)

    _d = Src0 + Src1
    _nd = Bin(AluOp.BITWISE_NOT, _d, _d)
    _s = _nd * C0
    seed_spec = Spec(body=_s * (C1 - _d * _s), reference=_ref_seed)
    _y2 = Src1 * (C0 - Src0 * Src1)
    nr2_spec = Spec(body=_y2 * (C0 - Src0 * _y2), reference=_ref_nr2)

    ops = []
    next_row = max(dve_ops_mod._SUB_OPCODE_FOR_NAME.values()) + 1
    for name, spec in (("ADD_RECIP_SEED_LIQ", seed_spec),
                       ("RECIP_NR2_LIQ", nr2_spec)):
        shas = {}
        for ver in ("v3", "v4"):
            try:
                compiled = DveOpSpec(name=name, opcode=next_row,
                                     uops=lower(spec, ver=ver),
                                     rd1_en=has_src1(spec))
                shas[ver] = compiled.sha(ver)
            except Exception:
                pass
        op = DveOp(name, spec, subdim=False, uops_sha=shas)
        dve_ops_mod.OPS.append(op)
        dve_ops_mod.CUSTOM_DVE_SPECS[name] = spec
        dve_ops_mod._SUB_OPCODE_FOR_NAME[name] = next_row
        next_row += 1
        ops.append(op)
    assert next_row <= 0x20, "custom DVE row overflow"
    dve_ops_mod.CUSTOM_DVE_OPS_LIQ = tuple(ops)  # type: ignore[attr-defined]
    return tuple(ops)


def _bf16_split(arr, terms):
    import ml_dtypes
    out = []
    rem = np.asarray(arr, dtype=np.float32).copy()
    for _ in range(terms):
        h = rem.astype(ml_dtypes.bfloat16)
        out.append(np.ascontiguousarray(h))
        rem = rem - h.astype(np.float32)
    return out


def _hi_view(ap):
    p, n = ap.shape
    return ap.bitcast(bf16).rearrange("p (n two) -> p n two", two=2)[:, :, 1]


def build(t_run=T, mm_mode="split3", recip=None, junk=None):
    assert mm_mode == "split3"
    recip = recip or RECIP
    junk = JUNK_MM if junk is None else junk
    if recip == "fused2":
        seed_op, nr2_op = _register_custom_recip_ops()
    t0 = _time.time()
    nc = bacc.Bacc("TRN2", target_bir_lowering=False, debug=False,
                   disable_frame_to_traceback=True)

    W = NCH * BLOC   # 64: merged state width
    TB = min(32, t_run)
    assert t_run % TB == 0
    NBLK = t_run // TB

    w_d = [nc.dram_tensor(f"wrec{j}", [128, NCH * NCH * 128], bf16,
                          kind="ExternalInput").ap() for j in range(2)]
    win_d = nc.dram_tensor("win", [128, FCH * NCH * 128], f32,
                           kind="ExternalInput").ap()
    it_d = nc.dram_tensor("it", [128, FCH * t_run * BLOC], f32,
                          kind="ExternalInput").ap()
    rtb_d = nc.dram_tensor("rtb", [128, t_run * BLOC], f32,
                           kind="ExternalInput").ap()
    invtau_d = nc.dram_tensor("invtau", [128, NCH], f32,
                              kind="ExternalInput").ap()
    bvec_d = nc.dram_tensor("bvec", [1, NCH * 128], f32,
                            kind="ExternalInput").ap()
    yout_d = nc.dram_tensor("yout", [t_run, 128, NCH * BLOC], f32,
                            kind="ExternalOutput").ap()

    with tile.TileContext(nc) as tc:
        import contextlib
        ctx = contextlib.ExitStack()
        with ctx:
            consts = ctx.enter_context(tc.tile_pool(name="consts", bufs=1))
            # state bufs=8: the output DMA reads the k5 y-tile; a deeper
            # rotation keeps its WAR guard pre-satisfied at the t boundary
            state = ctx.enter_context(tc.tile_pool(name="state", bufs=8))
            work = ctx.enter_context(tc.tile_pool(name="work", bufs=6))
            prep = ctx.enter_context(tc.tile_pool(name="prep", bufs=4))
            psum = ctx.enter_context(tc.tile_pool(name="psum", bufs=1, space="PSUM"))

            w_sb = []
            for j in range(2):
                wj = consts.tile([128, NCH * NCH * 128], bf16, name=f"w_sb{j}")
                nc.sync.dma_start(wj[:], w_d[j][:])
                w_sb.append(wj)
            win_sb = consts.tile([128, FCH * NCH * 128], f32, name="win_sb")
            nc.sync.dma_start(win_sb[:], win_d[:])
            it_sb = consts.tile([128, FCH * t_run * BLOC], f32, name="it_sb")
            nc.sync.dma_start(it_sb[:], it_d[:])
            rtb_sb = consts.tile([128, t_run * BLOC], f32, name="rtb_sb")
            nc.sync.dma_start(rtb_sb[:], rtb_d[:])
            invtau_sb = consts.tile([128, NCH], f32)
            nc.sync.dma_start(invtau_sb[:], invtau_d[:])
            bvec_sb = consts.tile([1, NCH * 128], f32)
            nc.sync.dma_start(bvec_sb[:], bvec_d[:])
            ones_sb = consts.tile([1, TB * BLOC], f32)
            nc.vector.memset(ones_sb[:], 1.0)
            junk1 = consts.tile([1, W], bf16)
            nc.vector.memset(junk1[:], 0.0)
            junk2 = consts.tile([1, 128], bf16)
            nc.vector.memset(junk2[:], 0.0)
            inp_sb = consts.tile([128, t_run * NCH * BLOC], f32, name="inp_sb")

            # PSUM: z banks (2 sets, merged [128, W]) + GEMM/junk banks
            zb = [psum.tile([128, W], f32, name=f"z{s}", tag=f"z{s}")
                  for s in range(2)]
            pg = [psum.tile([128, TB * BLOC], f32, name=f"pg{q}", tag=f"pg{q}")
                  for q in range(2)]

            _dve_chain = [None]
            _pe_chain = [None]
            _act_chain = [None]
            _gps_chain = [None]

            def _chain(slot, op, why):
                if slot[0] is not None:
                    add_dep_helper(op.ins, slot[0].ins, sync=False, reason=why)
                slot[0] = op
                return op

            def chain_dve(op):
                return _chain(_dve_chain, op, "DVE order")

            def chain_mm(op):
                return _chain(_pe_chain, op, "PE order")

            def chain_act(op):
                return _chain(_act_chain, op, "ACT order")

            def chain_gps(op):
                return _chain(_gps_chain, op, "GPS order")

            # ---- input projection GEMM: inp = i @ Win.T + b ----
            def win_tile(fc, mc):
                off = (fc * NCH + mc) * 128
                return win_sb[:, off:off + 128]

            for mc in range(NCH):
                for tb in range(NBLK):
                    bank = pg[(mc * NBLK + tb) % 2]
                    for fc in range(FCH):
                        base = fc * t_run * BLOC + tb * TB * BLOC
                        chain_mm(nc.tensor.matmul(
                            bank[:],
                            lhsT=win_tile(fc, mc),
                            rhs=it_sb[:, base:base + TB * BLOC],
                            start=(fc == 0), stop=False,
                            skip_group_check=True))
                    chain_mm(nc.tensor.matmul(
                        bank[:],
                        lhsT=bvec_sb[:, mc * 128:(mc + 1) * 128],
                        rhs=ones_sb[:],
                        start=False, stop=True,
                        skip_group_check=True))
                    dst = inp_sb[:].rearrange(
                        "p (t m b) -> p t m b", t=t_run, m=NCH)[
                        :, tb * TB:(tb + 1) * TB, mc, :]
                    chain_act(nc.scalar.activation(
                        dst, bank[:].rearrange("p (t b) -> p t b", t=TB),
                        mybir.ActivationFunctionType.Copy))

            # arm has_written bits of the z banks once
            for s in range(2):
                chain_mm(nc.tensor.matmul(
                    zb[s][:], lhsT=junk2[:], rhs=junk1[:],
                    start=True, stop=True))

            # ---- state: one two-slot tile ----
            def slot0(yk):
                return yk[:].rearrange("p (u b) -> p u b", u=2)[:, 0, :]

            def slot1_hi(yk):
                v = yk[:].bitcast(bf16).rearrange(
                    "p (u b two) -> p u b two", u=2, two=2)
                return v[:, 1, :, 1]

            def fused_rhs(yk, kc):
                v = yk[:].bitcast(bf16).rearrange(
                    "p (u b two) -> p u b two", u=2, two=2)[:, :, :, 1]
                return v[:, :, kc * BLOC:(kc + 1) * BLOC]

            yk_cur = state.tile([128, 2 * W], f32, name="y_init", tag="y")
            nc.vector.memset(yk_cur[:], 0.0)
            u_cur = work.tile([128, W], f32, name="u_init", tag="u")
            nc.vector.memset(u_cur[:], 0.0)

            def rt_slice(t):
                return rtb_sb[:, t * BLOC:(t + 1) * BLOC]

            def mk_rt_exp(t):
                rte = prep.tile([128, W], f32, tag="rte", name=f"rte{t}")
                chain_dve(nc.vector.tensor_copy(
                    rte[:].rearrange("p (m b) -> p m b", m=NCH),
                    rt_slice(t).unsqueeze(1).broadcast_to([128, NCH, BLOC])))
                return rte

            def mk_p2(t, rte):
                p2 = prep.tile([128, W], f32, tag="p2", name=f"p2_{t}")
                chain_dve(nc.vector.tensor_add(
                    p2[:].rearrange("p (m b) -> p m b", m=NCH),
                    rte[:].rearrange("p (m b) -> p m b", m=NCH),
                    invtau_sb[:, :].unsqueeze(2).broadcast_to([128, NCH, BLOC]),
                ))
                return p2

            def preload(bank, t2):
                src = inp_sb[:, t2 * W:(t2 + 1) * W]
                chain_act(nc.scalar.activation(
                    bank[:], src,
                    mybir.ActivationFunctionType.Copy))

            rte_cur = mk_rt_exp(0)
            p2_cur = mk_p2(0, rte_cur)
            preload(zb[0], 0)
            # unfold precision schedule: k0-k2 recompute z = W@y + inp in
            # full split3 (32 tiles); k3-k5 accumulate z += w0 @ hi16(dy)
            # (16 tiles) onto the PSUM-persistent z -- dy = y_k - y_{k-1} is
            # ~4% of |y| so a single bf16 pass holds the error budget.
            MODE = ("full", "full", "full", "single", "single", "single")

            def w_tile(j, kc, mc):
                off = (kc * NCH + mc) * 128
                return w_sb[j][:, off:off + 128]

            junk_rhs = w_sb[0][:, 0:512]

            def emit_junk():
                for _ in range(junk):
                    chain_mm(nc.tensor.matmul(
                        pg[0][:], lhsT=w_tile(0, 0, 0), rhs=junk_rhs,
                        start=True, stop=True, skip_group_check=True))

            rte_nxt = None
            p2_nxt = None
            dlt_cur = None
            total_gk = t_run * KUNF
            for t in range(t_run):
                for k in range(KUNF):
                    gk = t * KUNF + k
                    last_unfold = (k == KUNF - 1)
                    bank = zb[t % 2]
                    mode = MODE[k]
                    next_mode = MODE[(k + 1) % KUNF]

                    # re-preload inp for the k1/k2 full recomputes (ACT chain
                    # puts the copy after tanh(k-1)); t+1's bank at k==4
                    if k in (1, 2):
                        preload(bank, t)
                    if k == 4 and t + 1 < t_run:
                        preload(zb[(t + 1) % 2], t + 1)

                    # allocate all epilogue tiles up-front so their pool
                    # WAR-guard sems dispatch early and pre-satisfy
                    f_t = work.tile([128, W], f32, name="f", tag="f")
                    d_t = work.tile([128, W], f32, name="d", tag="d")
                    r_t = work.tile([128, W], f32, name="r", tag="r")
                    scr = work.tile([128, W], f32, name="rs", tag="rs")
                    nm_t = work.tile([128, W], f32, name="nm", tag="nm")
                    yk_new = state.tile([128, 2 * W], f32, name="y_n", tag="y")
                    u_new = work.tile([128, W], f32, name="u_n", tag="u")

                    # ---- MM stream ----
                    cnt = 0
                    total = 2 * NCH * NCH if mode == "full" else NCH * NCH
                    first_mm = None
                    last_mm = None

                    def emit(mm):
                        nonlocal first_mm, last_mm
                        if first_mm is None:
                            first_mm = mm
                        last_mm = mm

                    if mode == "full":
                        hv0 = _hi_view(slot0(yk_cur))
                        for kc in range(NCH):
                            for mc in range(NCH):
                                cnt += 1
                                emit(nc.tensor.matmul(
                                    bank[:, mc * BLOC:(mc + 1) * BLOC],
                                    lhsT=w_tile(1, kc, mc),
                                    rhs=hv0[:, kc * BLOC:(kc + 1) * BLOC],
                                    start=False, stop=(cnt == total),
                                    skip_group_check=True))
                        for kc in range(NCH):
                            frhs = fused_rhs(yk_cur, kc)
                            for mc in range(NCH):
                                cnt += 1
                                emit(nc.tensor.matmul(
                                    bank[:, mc * BLOC:(mc + 1) * BLOC]
                                    .unsqueeze(1).broadcast_to([128, 2, BLOC]),
                                    lhsT=w_tile(0, kc, mc),
                                    rhs=frhs,
                                    start=False, stop=(cnt == total),
                                    skip_group_check=True))
                    else:
                        hvD = _hi_view(dlt_cur[:])
                        for kc in range(NCH):
                            for mc in range(NCH):
                                cnt += 1
                                emit(nc.tensor.matmul(
                                    bank[:, mc * BLOC:(mc + 1) * BLOC],
                                    lhsT=w_tile(0, kc, mc),
                                    rhs=hvD[:, kc * BLOC:(kc + 1) * BLOC],
                                    start=False, stop=(cnt == total),
                                    skip_group_check=True))
                    if _pe_chain[0] is not None:
                        add_dep_helper(first_mm.ins, _pe_chain[0].ins,
                                       sync=False, reason="PE ordering")
                    _pe_chain[0] = last_mm

                    # ---- merged epilogue ----
                    chain_act(nc.scalar.activation(
                        f_t[:], bank[:],
                        mybir.ActivationFunctionType.Tanh))

                    if recip == "fused2":
                        # pool WAR-guard lands on this early touch (runs in
                        # the tanh window) instead of stalling the mul
                        chain_dve(nc.vector.memset(yk_new[:, W:W + 1], 0.0))
                        # DVE: seed(f+p2) -> [nm fills d-wait] -> NR2 -> mul
                        # GPS computes d = f + p2 in parallel for the NR.
                        chain_gps(nc.gpsimd.tensor_add(
                            d_t[:], f_t[:], p2_cur[:]))
                        chain_dve(nc.vector._custom_dve(
                            seed_op, out=scr[:], in0=f_t[:], in1=p2_cur[:],
                            s0=-0.23549792, s1=2.0017324))
                        chain_dve(nc.vector.tensor_add(
                            nm_t[:], f_t[:], u_cur[:]))
                        chain_dve(nc.vector._custom_dve(
                            nr2_op, out=r_t[:], in0=d_t[:], in1=scr[:],
                            s0=2.0))
                    else:
                        chain_dve(nc.vector.tensor_add(
                            d_t[:], f_t[:], p2_cur[:]))
                        if recip == "accurate":
                            chain_dve(nc.vector.reciprocal(
                                out=r_t[:], in_=d_t[:]))
                        else:
                            chain_dve(nc.vector.reciprocal_approx_fast(
                                out=scr[:], in_=d_t[:]))
                            from concourse.dve_ops import RECIPROCAL_APPROX_NR
                            chain_dve(nc.vector._custom_dve(
                                RECIPROCAL_APPROX_NR, out=r_t[:], in0=d_t[:],
                                in1=scr[:], s0=2.0))
                        chain_gps(nc.gpsimd.tensor_add(
                            nm_t[:], f_t[:], u_cur[:]))
                    # split mul (and the follow-up sub) into kc-halves: the
                    # next stream's kc0-1 MMs gate on the _a half only
                    H = W // 2
                    s0n = slot0(yk_new)
                    s1h = slot1_hi(yk_new)
                    s0o = slot0(yk_cur)
                    if next_mode == "single":
                        dlt_new = work.tile([128, W], f32, name="dd", tag="dd")
                        chain_dve(nc.vector.tensor_mul(
                            s0n[:, 0:H], nm_t[:, 0:H], r_t[:, 0:H]))
                        chain_dve(nc.vector.tensor_sub(
                            dlt_new[:, 0:H], s0n[:, 0:H], s0o[:, 0:H]))
                        chain_dve(nc.vector.tensor_mul(
                            s0n[:, H:], nm_t[:, H:], r_t[:, H:]))
                        chain_dve(nc.vector.tensor_sub(
                            dlt_new[:, H:], s0n[:, H:], s0o[:, H:]))
                        dlt_cur = dlt_new
                    else:
                        chain_dve(nc.vector.tensor_mul(
                            s0n[:, 0:H], nm_t[:, 0:H], r_t[:, 0:H]))
                        chain_dve(nc.vector.tensor_mul(
                            s0n[:, H:], nm_t[:, H:], r_t[:, H:]))
                        chain_gps(nc.gpsimd.tensor_sub(
                            s1h[:, 0:H], s0n[:, 0:H], _hi_view(s0n)[:, 0:H]))
                        chain_gps(nc.gpsimd.tensor_sub(
                            s1h[:, H:], s0n[:, H:], _hi_view(s0n)[:, H:]))
                    if not last_unfold:
                        chain_gps(nc.gpsimd.tensor_mul(
                            u_new[:], s0n, rte_cur[:]))
                    elif t + 1 < t_run:
                        chain_gps(nc.gpsimd.tensor_mul(
                            u_new[:], s0n, rte_nxt[:]))

                    if gk + 1 < total_gk:
                        emit_junk()

                    if k == 2 and t + 1 < t_run:
                        rte_nxt = mk_rt_exp(t + 1)
                        p2_nxt = mk_p2(t + 1, rte_nxt)
                    yk_cur = yk_new
                    u_cur = u_new

                # stage the output through an ACT copy so the slow DMA reads
                # a staging tile, not the y state tile — keeps the y pool's
                # recycle guards off the DMA completion chain
                st = prep.tile([128, W], f32, name=f"st{t}", tag="st")
                chain_act(nc.scalar.activation(
                    st[:], slot0(yk_cur),
                    mybir.ActivationFunctionType.Copy))
                nc.sync.dma_start(yout_d[t][:, :], st[:])
                if t + 1 < t_run:
                    rte_cur, p2_cur = rte_nxt, p2_nxt

    t1 = _time.time()
    nc.compile()
    t2 = _time.time()
    if VERBOSE:
        print(f"[build] trace+schedule {t1-t0:.1f}s, bacc compile {t2-t1:.1f}s",
              flush=True)
    return nc


def _host_prep(i, delta_t, W_rec, W_in, b, A, tau, t_run):
    i = np.asarray(i, dtype=np.float32)
    delta_t = np.asarray(delta_t, dtype=np.float32)
    W_rec = np.asarray(W_rec, dtype=np.float32)
    W_in = np.asarray(W_in, dtype=np.float32)
    b = np.asarray(b, dtype=np.float32)
    A = np.asarray(A, dtype=np.float32)
    tau = np.asarray(tau, dtype=np.float32)

    def tiles_rec(m):
        return m.reshape(NCH, 128, NCH, 128).transpose(1, 0, 2, 3).reshape(128, -1)

    def tiles_in(m):
        return m.reshape(FCH, 128, NCH, 128).transpose(1, 0, 2, 3).reshape(128, -1)

    Wt = (W_rec * A[None, :]).T
    w_arrs = [np.ascontiguousarray(tiles_rec(x.astype(np.float32)).astype(x.dtype))
              for x in _bf16_split(Wt, 2)]
    win_arr = np.ascontiguousarray(tiles_in(W_in.T), dtype=np.float32)

    invtau = np.ascontiguousarray((1.0 / tau).reshape(NCH, 128).T, dtype=np.float32)
    bvec = np.ascontiguousarray(b.reshape(1, -1), dtype=np.float32)

    in_maps = []
    for c in range(NCORES):
        bsl = slice(c * BLOC, (c + 1) * BLOC)
        ii = i[bsl, :t_run]
        it = np.ascontiguousarray(
            ii.reshape(BLOC, t_run, FCH, 128).transpose(3, 2, 1, 0)
            .reshape(128, -1), dtype=np.float32)
        r = (KUNF / np.maximum(delta_t[bsl, :t_run], 1e-30)).T.reshape(1, -1)
        rtb = np.ascontiguousarray(
            np.broadcast_to(r, (128, r.shape[1])), dtype=np.float32)
        m = {"it": it, "rtb": rtb, "invtau": invtau, "bvec": bvec,
             "win": win_arr, "wrec0": w_arrs[0], "wrec1": w_arrs[1]}
        in_maps.append(m)
    return in_maps


def _host_unshard(results, A, t_run):
    A = np.asarray(A, dtype=np.float32)
    out = np.empty((B, t_run, N), dtype=np.float32)
    for c in range(NCORES):
        y = results[c]["yout"].reshape(t_run, 128, NCH, BLOC)
        xc = y.transpose(3, 0, 2, 1).reshape(BLOC, t_run, N)
        out[c * BLOC:(c + 1) * BLOC] = xc * A[None, None, :]
    return out


_BUILD_CACHE = {}


def _get_built(t_run, mm_mode):
    key = (t_run, mm_mode)
    if key not in _BUILD_CACHE:
        _BUILD_CACHE[key] = build(t_run, mm_mode)
    return _BUILD_CACHE[key]


def run(i, delta_t, W_rec, W_in, b, A, tau, t_run=T, mm_mode="split3",
        **rb_kwargs):
    nc = _get_built(t_run, mm_mode)
    in_maps = _host_prep(i, delta_t, W_rec, W_in, b, A, tau, t_run)
    res = run_bass_kernel_spmd(nc, in_maps, list(range(NCORES)), **rb_kwargs)
    out = _host_unshard(res.results, A, t_run)
    return out, res


MM_DTYPE = "split3"


def kernel(i, delta_t, W_rec, W_in, b, A, tau):
    out, _ = run(i, delta_t, W_rec, W_in, b, A, tau)
    return out
